# revision 1
# baseline (speedup 1.0000x reference)
"""SchNet-style GNN message passing on 8 Trainium2 NeuronCores.

Strategy (per sharding hint): edges sharded by destination atom across 8
cores; atoms relabeled + degree-balanced so each core owns an equal shard
of destination atoms, with edges padded into a fixed, SPMD-uniform static
schedule.  Small weights replicated.  Per conv: the updated atom-state
shards are exchanged with one AllGather; per-edge source features are
fetched with hardware gather-DMA from a per-core hf table; the scatter-add
(segment sum) is done on the tensor engine as one-hot matmuls into PSUM
accumulators (edges pre-sorted by destination chunk).  Final per-molecule
energies come from a mask matmul; host sums the 8 partial [n_mol] vectors.
"""

import os
import sys
import numpy as np

KSTAGE = int(os.environ.get("KSTAGE", "4"))

sys.path.insert(0, "/opt/trn_rl_repo")

from contextlib import ExitStack

import ml_dtypes
import concourse.bass as bass
import concourse.tile as tile
import concourse.bacc as bacc
from concourse import mybir
from concourse import bass_utils

F32 = mybir.dt.float32
BF16 = mybir.dt.bfloat16
I16 = mybir.dt.int16
AF = mybir.ActivationFunctionType
OP = mybir.AluOpType

LN2 = float(np.log(2.0))
EPS = 1e-12
P = 128          # partitions / chunk size
NG = 32          # gaussians
NB = 128         # atom basis / filters
NH = 64          # readout hidden


# ----------------------------------------------------------------------------
# Host-side plan: atom relabeling, edge sharding, static schedule
# ----------------------------------------------------------------------------

class Plan:
    pass


def _greedy_pack(deg_a, deg_b, atom_ids, n_bins, rng):
    """Pack len(atom_ids) atoms into n_bins bins of exactly P atoms each,
    balancing per-bin sums of deg_a and deg_b.  Returns [n_bins, P] atom ids
    (-1 for none -> caller guarantees exact fit)."""
    n = len(atom_ids)
    assert n == n_bins * P
    tot_a = max(float(deg_a[atom_ids].sum()), 1.0)
    tot_b = max(float(deg_b[atom_ids].sum()), 1.0)
    ta = tot_a / n_bins
    tb = tot_b / n_bins
    order = np.argsort(-(deg_a[atom_ids] + deg_b[atom_ids]), kind="stable")
    sa = np.zeros(n_bins)
    sb = np.zeros(n_bins)
    cnt = np.zeros(n_bins, dtype=np.int64)
    bins = np.full((n_bins, P), -1, dtype=np.int64)
    for oi in order:
        a = atom_ids[oi]
        da, db = deg_a[a], deg_b[a]
        load = np.maximum((sa + da) / ta, (sb + db) / tb)
        load[cnt >= P] = np.inf
        i = int(np.argmin(load))
        bins[i, cnt[i]] = a
        cnt[i] += 1
        sa[i] += da
        sb[i] += db
    assert (cnt == P).all()
    return bins, sa, sb


def make_plan(r, xyz, a, n_per, n_cores=8):
    pl = Plan()
    n_atoms = xyz.shape[0]
    n_edges = a.shape[0]
    rng = np.random.default_rng(12345)

    # padded atom count: multiple of n_cores*P
    npad = ((n_atoms + n_cores * P - 1) // (n_cores * P)) * (n_cores * P)
    K = npad // (n_cores * P)          # bins (chunks) per core
    SH = K * P                          # atoms per core shard
    a_cap = min(32768, npad)            # atoms addressable by table A (idx16)
    a_cap = (a_cap // P) * P
    if a_cap < npad // 2:
        raise ValueError("atom table too large for two-base gather split")
    b_base = npad - a_cap               # B gather base row

    dst = a[:, 0].astype(np.int64)
    src = a[:, 1].astype(np.int64)

    # choose A-set (atoms whose NEW id < a_cap): random choice of a_cap reals;
    # virtual atoms (degree 0) fill whatever space remains in each group.
    n_virt = npad - n_atoms
    n_aset = min(a_cap, n_atoms)
    perm_r = rng.permutation(n_atoms)
    aset = np.zeros(n_atoms, dtype=bool)
    aset[perm_r[:n_aset]] = True

    in_a = aset[src]                    # edge half by src membership
    degA = np.bincount(dst[in_a], minlength=n_atoms)
    degB = np.bincount(dst[~in_a], minlength=n_atoms)
    degA_x = np.concatenate([degA, np.zeros(n_virt, dtype=degA.dtype)])
    degB_x = np.concatenate([degB, np.zeros(n_virt, dtype=degB.dtype)])

    virt_ids = np.arange(n_atoms, npad)
    n_virt_a = a_cap - n_aset            # virtuals needed in the A group
    a_ids = np.concatenate([np.nonzero(aset)[0], virt_ids[:n_virt_a]])
    b_ids_x = np.concatenate([np.nonzero(~aset)[0], virt_ids[n_virt_a:]])
    binsA, saA, sbA = _greedy_pack(degA_x, degB_x, a_ids, a_cap // P, rng)
    if npad > a_cap:
        binsB, saB, sbB = _greedy_pack(degA_x, degB_x, b_ids_x,
                                       (npad - a_cap) // P, rng)
    else:
        binsB = np.zeros((0, P), dtype=np.int64)
        saB = np.zeros(1)
        sbB = np.zeros(1)

    new_of_old = np.full(npad, -1, dtype=np.int64)
    allbins = np.concatenate([binsA, binsB], axis=0)    # [npad//P, P]
    flat = allbins.reshape(-1)
    new_of_old[flat] = np.arange(npad)
    old_of_new = flat                                    # new id -> old id

    maxA = int(np.maximum(saA.max(), saB.max()))
    maxB = int(np.maximum(sbA.max(), sbB.max()))
    TA = ((maxA + P - 1) // P + 3) // 4 * 4              # subtiles, mult of 4
    TA = max(TA, 4)
    TB = ((maxB + P - 1) // P + 1) // 2 * 2              # mult of 2
    TB = max(TB, 2)

    # static stream structure (identical for every core)
    groups = [(c, c + 1) for c in range(0, K - 1, 2)]
    if K % 2 == 1:
        groups.append((K - 1,))
    n_sub_main = sum(len(g) * (TA + TB) for g in groups)
    padb = (16 - (n_sub_main % 16)) % 16                # pad to 2048-edge mult
    # also need last B span % 4 == 0
    lastB = len(groups[-1]) * TB + padb
    while lastB % 4 != 0:
        padb += 16
        lastB = len(groups[-1]) * TB + padb
    n_sub = n_sub_main + padb
    Ep = n_sub * P

    # per-subtile chunk binding + gather call table (same for all cores)
    st_chunk = np.zeros(n_sub, dtype=np.int64)
    calls = []            # (start_subtile, n_subtiles, half)  half: 0=A 1=B
    sub_slots = []        # per group/half: list of (chunk, n_subtiles)
    s = 0
    for gi, g in enumerate(groups):
        st0 = s
        for c in g:
            st_chunk[s:s + TA] = c
            s += TA
        calls.append((st0, s - st0, 0))
        st0 = s
        for c in g:
            st_chunk[s:s + TB] = c
            s += TB
        if gi == len(groups) - 1 and padb:
            st_chunk[s:s + padb] = g[-1]
            s += padb
        calls.append((st0, s - st0, 1))
    assert s == n_sub
    # one dma_gather per span (multi-packet mode); keep calls <= 32 subtiles
    calls2 = []
    for (st0, ns, half) in calls:
        off = 0
        while off < ns:
            take = min(32, ns - off)
            calls2.append((st0 + off, take, half))
            off += take
    calls = calls2
    max_call_sub = max(ns for _, ns, _ in calls)

    # last subtile index of each chunk (for psum close / stop flag)
    chunk_last = np.zeros(K, dtype=np.int64)
    chunk_first = np.zeros(K, dtype=np.int64)
    seen = set()
    for st in range(n_sub):
        c = st_chunk[st]
        if c not in seen:
            chunk_first[c] = st
            seen.add(c)
        chunk_last[c] = st

    # ---- per-core edge data -------------------------------------------------
    src_new = new_of_old[src]
    dst_new = new_of_old[dst]
    e_core = dst_new // SH
    e_chunk = (dst_new % SH) // P
    e_half = (src_new >= a_cap).astype(np.int64)

    idx_lin = np.zeros((n_cores, Ep), dtype=np.int16)
    dstrel_lin = np.full((n_cores, Ep), -1.0, dtype=np.float32)
    osrc_lin = np.zeros((n_cores, Ep), dtype=np.int64)
    odst_lin = np.zeros((n_cores, Ep), dtype=np.int64)

    # bucket edges by (core, chunk, half)
    order = np.lexsort((e_half, e_chunk, e_core))
    so_src, so_dst = src_new[order], dst_new[order]
    so_core, so_chunk, so_half = e_core[order], e_chunk[order], e_half[order]
    so_osrc, so_odst = src[order], dst[order]
    # boundaries
    keys = so_core * (K * 2) + so_chunk * 2 + so_half
    bstart = np.searchsorted(keys, np.arange(n_cores * K * 2), side="left")
    bend = np.searchsorted(keys, np.arange(n_cores * K * 2), side="right")

    # subtile start position for each (chunk, half) within the stream
    span_start = {}
    s = 0
    for gi, g in enumerate(groups):
        for c in g:
            span_start[(c, 0)] = s
            s += TA
        for c in g:
            span_start[(c, 1)] = s
            s += TB
        if gi == len(groups) - 1:
            s += padb

    for core in range(n_cores):
        for c in range(K):
            for h in (0, 1):
                bi = core * (K * 2) + c * 2 + h
                e0, e1 = bstart[bi], bend[bi]
                cnt = e1 - e0
                cap = (TA if h == 0 else TB) * P
                assert cnt <= cap, (core, c, h, cnt, cap)
                p0 = span_start[(c, h)] * P
                sl = slice(p0, p0 + cnt)
                if h == 0:
                    idx_lin[core, sl] = so_src[e0:e1].astype(np.int16)
                else:
                    idx_lin[core, sl] = (so_src[e0:e1] - b_base).astype(np.int16)
                dstrel_lin[core, sl] = (so_dst[e0:e1] % P).astype(np.float32)
                osrc_lin[core, sl] = so_osrc[e0:e1]
                odst_lin[core, sl] = so_odst[e0:e1]

    pl.n_atoms, pl.n_edges, pl.npad = n_atoms, n_edges, npad
    pl.n_cores, pl.K, pl.SH, pl.Ep, pl.n_sub = n_cores, K, SH, Ep, n_sub
    pl.TA, pl.TB, pl.padb = TA, TB, padb
    pl.a_cap, pl.b_base = a_cap, b_base
    pl.groups, pl.calls, pl.max_call_sub = groups, calls, max_call_sub
    pl.st_chunk, pl.chunk_first, pl.chunk_last = st_chunk, chunk_first, chunk_last
    pl.new_of_old, pl.old_of_new = new_of_old, old_of_new
    pl.idx_lin, pl.dstrel_lin = idx_lin, dstrel_lin
    pl.osrc_lin, pl.odst_lin = osrc_lin, odst_lin
    pl.n_per = int(n_per)
    pl.n_mol = n_atoms // pl.n_per
    return pl


def make_inputs(pl, r, xyz, a, embed, weights):
    """Build per-core in_maps.  weights: dict of raw weight arrays."""
    C, K, SH, Ep, n_sub = pl.n_cores, pl.K, pl.SH, pl.Ep, pl.n_sub
    NC = weights["fw1"].shape[0]
    NM = pl.n_mol
    F0 = Ep // P

    h0_all = embed[r[:, 0].astype(np.int64)].astype(np.float32)     # [n,NB]
    h0_new = np.zeros((pl.npad, NB), dtype=np.float32)
    real = pl.old_of_new < pl.n_atoms
    h0_new[real] = h0_all[pl.old_of_new[real]]

    mol_new = np.full(pl.npad, -1, dtype=np.int64)
    mol_new[real] = pl.old_of_new[real] // pl.n_per

    xyzf = xyz.astype(np.float32)

    fw1, fb1 = weights["fw1"], weights["fb1"]
    fw2, fb2 = weights["fw2"], weights["fb2"]
    afw, afb = weights["afw"], weights["afb"]
    ow1, ob1 = weights["ow1"], weights["ob1"]
    ow2, ob2 = weights["ow2"], weights["ob2"]
    aw1, ab1 = weights["aw1"], weights["ab1"]
    aw2, ab2 = weights["aw2"], weights["ab2"]
    assert np.all(afb == 0.0), "nonzero afb not supported by this kernel"

    # fold ssp's -log(2) into the following layer's bias
    fb2e = (fb2 - LN2 * fw2.sum(axis=1)).astype(np.float32)         # [NC,NB]
    ob2e = (ob2 - LN2 * ow2.sum(axis=1)).astype(np.float32)         # [NC,NB]
    ab2e = float(ab2[0] - LN2 * aw2.sum(axis=0)[0])

    offs = np.linspace(0.0, 5.0, NG).astype(np.float32)
    width = float(offs[1] - offs[0])
    coeff = -0.5 / (width * width)

    shared = {
        "fw1b": np.ascontiguousarray(
            fw1.transpose(1, 0, 2).reshape(NG, NC * NB)).astype(ml_dtypes.bfloat16),
        "fw2b": np.ascontiguousarray(
            fw2.transpose(1, 0, 2).reshape(NB, NC * NB)).astype(ml_dtypes.bfloat16),
        "afwb": np.ascontiguousarray(
            afw.transpose(1, 0, 2).reshape(NB, NC * NB)).astype(ml_dtypes.bfloat16),
        "ow1w": np.ascontiguousarray(
            ow1.transpose(1, 0, 2).reshape(NB, NC * NB)).astype(np.float32),
        "ow2w": np.ascontiguousarray(
            ow2.transpose(1, 0, 2).reshape(NB, NC * NB)).astype(np.float32),
        "aw1w": aw1.astype(np.float32),                              # [NB,NH]
        "aw2w": aw2.astype(np.float32),                              # [NH,1]
        "fb1t": np.ascontiguousarray(fb1.T).astype(np.float32),      # [NB,NC]
        "ob1t": np.ascontiguousarray(ob1.T).astype(np.float32),
        "ob2et": np.ascontiguousarray(ob2e.T).astype(np.float32),
        "fb2e4": np.ascontiguousarray(np.tile(fb2e, (1, 4)).reshape(1, -1)).astype(np.float32),
        "ab1t": ab1.reshape(NH, 1).astype(np.float32),
        "ab2p": np.full((P, 1), ab2e, dtype=np.float32),
        "negmu": np.tile(-offs, 4).reshape(P, 1).astype(np.float32),
        "iotaf": np.tile(np.arange(P, dtype=np.float32), (P, 8)),
    }

    in_maps = []
    for c in range(C):
        m = dict(shared)
        osrc = pl.osrc_lin[c]
        odst = pl.odst_lin[c]
        xs = xyzf[osrc]          # [Ep,3]
        xd = xyzf[odst]
        m["xsx"] = np.ascontiguousarray(xs[:, 0].reshape(P, F0))
        m["xsy"] = np.ascontiguousarray(xs[:, 1].reshape(P, F0))
        m["xsz"] = np.ascontiguousarray(xs[:, 2].reshape(P, F0))
        m["xdx"] = np.ascontiguousarray(xd[:, 0].reshape(P, F0))
        m["xdy"] = np.ascontiguousarray(xd[:, 1].reshape(P, F0))
        m["xdz"] = np.ascontiguousarray(xd[:, 2].reshape(P, F0))
        m["idx"] = np.ascontiguousarray(
            np.tile(pl.idx_lin[c].reshape(Ep // 16, 16).T, (8, 1)))
        m["dstrel"] = np.ascontiguousarray(
            pl.dstrel_lin[c].reshape(n_sub, P).T)
        m["h0t"] = np.ascontiguousarray(
            h0_new[c * SH:(c + 1) * SH].T)                          # [NB,SH]
        msk = np.zeros((K, P, NM), dtype=np.float32)
        mols = mol_new[c * SH:(c + 1) * SH].reshape(K, P)
        for mm in range(NM):
            msk[:, :, mm] = (mols == mm)
        m["mask"] = msk
        in_maps.append(m)
    return in_maps, coeff


# ----------------------------------------------------------------------------
# Device program
# ----------------------------------------------------------------------------

def _ap(tile_ap, extra_off, pattern):
    """Raw access-pattern surgery on a (pool-tile or dram) AP."""
    return bass.AP(tile_ap.tensor, tile_ap.offset + extra_off, pattern)


def _patch_act_tables():
    """Confine Exp/Ln/Copy/Identity/Square to natural_log_exp_and_others so
    bacc doesn't thrash ACT table loads between alternating sets."""
    import concourse.hw_specs as hws
    if getattr(bacc, "_act_tables_patched", False):
        return
    orig = bacc.get_activation_tables

    def patched(arch):
        t = dict(orig(arch))
        shared = {AF.Exp, AF.Ln, AF.Identity, AF.Copy, AF.Square}
        for name in t:
            if name != "natural_log_exp_and_others":
                t[name] = t[name] - shared
        return t

    bacc.get_activation_tables = patched
    bacc._act_tables_patched = True


def build_program(pl, NC, NM, coeff):
    _patch_act_tables()
    C, K, SH, Ep, n_sub = pl.n_cores, pl.K, pl.SH, pl.Ep, pl.n_sub
    F0 = Ep // P
    Q = Ep // 4                      # edges per gaussian partition-group
    NW = 8                           # phase-0 g-build col iterations
    while Q % NW != 0 or (Q // NW) > 2048:
        NW *= 2
    Wg = Q // NW

    nc = bacc.Bacc("TRN2", target_bir_lowering=False, debug=False,
                   enable_asserts=False, num_devices=C, num_swdge_queues=4)

    def din(name, shape, dt=F32):
        return nc.dram_tensor(name, shape, dt, kind="ExternalInput").ap()

    xsx, xsy, xsz = din("xsx", [P, F0]), din("xsy", [P, F0]), din("xsz", [P, F0])
    xdx, xdy, xdz = din("xdx", [P, F0]), din("xdy", [P, F0]), din("xdz", [P, F0])
    idx_d = din("idx", [P, Ep // 16], I16)
    dstrel_d = din("dstrel", [P, n_sub])
    h0t_d = din("h0t", [NB, SH])
    mask_d = din("mask", [K, P, NM])
    fw1b_d = din("fw1b", [NG, NC * NB], BF16)
    fw2b_d = din("fw2b", [NB, NC * NB], BF16)
    afwb_d = din("afwb", [NB, NC * NB], BF16)
    ow1w_d = din("ow1w", [NB, NC * NB])
    ow2w_d = din("ow2w", [NB, NC * NB])
    aw1w_d = din("aw1w", [NB, NH])
    aw2w_d = din("aw2w", [NH, 1])
    fb1t_d = din("fb1t", [NB, NC])
    ob1t_d = din("ob1t", [NB, NC])
    ob2et_d = din("ob2et", [NB, NC])
    fb2e4_d = din("fb2e4", [1, NC * 4 * NB])
    ab1t_d = din("ab1t", [NH, 1])
    ab2p_d = din("ab2p", [P, 1])
    negmu_d = din("negmu", [P, 1])
    iotaf_d = din("iotaf", [P, 8 * P])

    ypart = nc.dram_tensor("ypart", [1, NM], F32, kind="ExternalOutput").ap()

    with tile.TileContext(nc) as tc:
        with ExitStack() as ctx:
            dram = ctx.enter_context(tc.tile_pool(name="dram", bufs=1, space="DRAM"))
            res = ctx.enter_context(tc.tile_pool(name="res", bufs=1))
            sb = ctx.enter_context(tc.tile_pool(name="sb", bufs=3))
            gpool = ctx.enter_context(tc.tile_pool(name="gpool", bufs=3))
            p0 = ctx.enter_context(tc.tile_pool(name="p0", bufs=1))
            spool = ctx.enter_context(tc.tile_pool(name="spool", bufs=3))
            pps = ctx.enter_context(tc.tile_pool(name="pps", bufs=2, space="PSUM"))
            ppagg = ctx.enter_context(tc.tile_pool(name="ppagg", bufs=2, space="PSUM"))
            ppsm = ppagg

            # ---- DRAM scratch ----
            d_dram = dram.tile([P, F0], F32)
            g_dram = dram.tile([4, NG, Q], BF16)
            hf_dram = dram.tile([pl.npad, NB], BF16)
            bounce = dram.tile([NB, SH], BF16)
            s_dram = dram.tile([P, n_sub * P], BF16)
            gath = dram.tile([C * NB, SH], BF16)

            # ---- resident SBUF ----
            h_my = res.tile([NB, SH], F32)
            agg_sb = res.tile([NB, SH], F32)
            idx_sb = res.tile([P, Ep // 16], I16)
            dstrel_sb = res.tile([P, n_sub], F32)
            iotaf_sb = res.tile([P, 8 * P], F32)
            mask_sb = res.tile([P, K * NM], F32)
            ones_sb = res.tile([1, P], F32)
            fw1b_sb = res.tile([NG, NC * NB], BF16)
            fw2b_sb = res.tile([NB, NC * NB], BF16)
            afwb_sb = res.tile([NB, NC * NB], BF16)
            ow1_sb = res.tile([NB, NC * NB], F32)
            ow2_sb = res.tile([NB, NC * NB], F32)
            aw1_sb = res.tile([NB, NH], F32)
            aw2_sb = res.tile([NH, 1], F32)
            fb1_sb = res.tile([NB, NC], F32)
            ob1_sb = res.tile([NB, NC], F32)
            ob2e_sb = res.tile([NB, NC], F32)
            fb2e4_sb = res.tile([1, NC * 4 * NB], F32)
            ab1_sb = res.tile([NH, 1], F32)
            ab2p_sb = res.tile([P, 1], F32)
            negmu_sb = res.tile([P, 1], F32)
            epsb_sb = res.tile([P, 1], F32)
            nc.gpsimd.memset(epsb_sb[:], EPS)

            nc.sync.dma_start(h_my[:], h0t_d[:])
            nc.sync.dma_start(idx_sb[:], idx_d[:])
            nc.sync.dma_start(dstrel_sb[:], dstrel_d[:])
            nc.sync.dma_start(iotaf_sb[:], iotaf_d[:])
            # mask [K,P,NM] -> [P, K*NM]
            nc.sync.dma_start(
                _ap(mask_sb[:], 0, [[K * NM, P], [NM, K], [1, NM]]),
                _ap(mask_d, 0, [[NM, P], [P * NM, K], [1, NM]]))
            nc.gpsimd.memset(ones_sb[:], 1.0)
            for t_sb, t_d in [(fw1b_sb, fw1b_d), (fw2b_sb, fw2b_d),
                              (afwb_sb, afwb_d), (ow1_sb, ow1w_d),
                              (ow2_sb, ow2w_d), (aw1_sb, aw1w_d),
                              (aw2_sb, aw2w_d), (fb1_sb, fb1t_d),
                              (ob1_sb, ob1t_d), (ob2e_sb, ob2et_d),
                              (fb2e4_sb, fb2e4_d), (ab1_sb, ab1t_d),
                              (ab2p_sb, ab2p_d), (negmu_sb, negmu_d)]:
                nc.sync.dma_start(t_sb[:], t_d[:])

            # ================= phase 0: distances and gaussians ============
            cx = p0.tile([P, F0], F32, tag="ph0")
            cy = p0.tile([P, F0], F32, tag="ph0b")
            cz = p0.tile([P, F0], F32, tag="ph0c")
            tx = p0.tile([P, F0], F32, tag="ph0d")
            nc.sync.dma_start(cx[:], xsx[:])
            nc.sync.dma_start(tx[:], xdx[:])
            nc.vector.tensor_sub(cx[:], cx[:], tx[:])
            nc.vector.tensor_mul(cx[:], cx[:], cx[:])
            nc.sync.dma_start(cy[:], xsy[:])
            nc.sync.dma_start(tx[:], xdy[:])
            nc.vector.tensor_sub(cy[:], cy[:], tx[:])
            nc.vector.tensor_mul(cy[:], cy[:], cy[:])
            nc.sync.dma_start(cz[:], xsz[:])
            nc.sync.dma_start(tx[:], xdz[:])
            nc.vector.tensor_sub(cz[:], cz[:], tx[:])
            nc.vector.tensor_mul(cz[:], cz[:], cz[:])
            nc.vector.tensor_add(cx[:], cx[:], cy[:])
            nc.vector.tensor_add(cx[:], cx[:], cz[:])
            nc.scalar.activation(cy[:], cx[:], AF.Sqrt,
                                 bias=epsb_sb[:, 0:1], scale=1.0)
            nc.sync.dma_start(d_dram[:], cy[:])

            for w in range(NW):
                dbc = p0.tile([P, Wg], F32, tag="dbc")
                nc.sync.dma_start(
                    dbc[:], _ap(d_dram[:], w * Wg, [[Q, 4], [0, NG], [1, Wg]]))
                t1 = p0.tile([P, Wg], F32, tag="t1")
                nc.scalar.activation(t1[:], dbc[:], AF.Square,
                                     bias=negmu_sb[:, 0:1], scale=1.0)
                gt = p0.tile([P, Wg], BF16, tag="gt0")
                nc.scalar.activation(gt[:], t1[:], AF.Exp, bias=0.0, scale=coeff)
                nc.sync.dma_start(
                    _ap(g_dram[:], w * Wg, [[NG * Q, 4], [Q, NG], [1, Wg]]),
                    gt[:])

            # ================= conv layers =================================
            for i in range(NC if KSTAGE >= 2 else 0):
                # --- allgather h, build hf table -----------------------
                nc.gpsimd.dma_start(bounce[:], h_my[:])        # f32 -> bf16
                nc.gpsimd.collective_compute(
                    "AllGather", OP.bypass,
                    replica_groups=[list(range(C))],
                    ins=[bounce.opt()], outs=[gath.opt()])
                n_cg = (SH + 511) // 512
                for s_sh in range(C):
                    for cg in range(n_cg):
                        c0 = cg * 512
                        w = min(512, SH - c0)
                        ht = sb.tile([NB, 512], BF16, tag="ht")
                        nc.sync.dma_start(ht[:, :w],
                                          gath[NB * s_sh:NB * (s_sh + 1),
                                               c0:c0 + w])
                        hfps = pps.tile([P, 512], F32, tag="ps2", name="hfps")
                        for c4 in range(w // P):
                            nc.tensor.matmul(
                                hfps[:, P * c4:P * (c4 + 1)],
                                ht[:, P * c4:P * (c4 + 1)],
                                afwb_sb[:, NB * i:NB * (i + 1)],
                                start=True, stop=True)
                        hfsb = sb.tile([P, 512], BF16, tag="hfsb")
                        if s_sh % 2 == 0:
                            nc.scalar.copy(hfsb[:, :w], hfps[:, :w])
                        else:
                            nc.vector.tensor_copy(hfsb[:, :w], hfps[:, :w])
                        row0 = s_sh * SH + c0
                        nc.sync.dma_start(
                            _ap(hf_dram[:], row0 * NB,
                                [[NB, P], [NB * P, w // P], [1, NB]]),
                            _ap(hfsb[:], 0, [[512, P], [P, w // P], [1, P]]))

                # --- edge phase ---------------------------------------
                agg_open = {}
                for ci, (st0, nsx, half) in enumerate(pl.calls if KSTAGE >= 3 else []):
                    gbuf = gpool.tile([P, pl.max_call_sub, NB], BF16, tag="gbuf")
                    if half == 0:
                        tbl_ap = _ap(hf_dram[:], 0, [[NB, pl.a_cap], [1, NB]])
                    else:
                        tbl_ap = _ap(hf_dram[:], pl.b_base * NB,
                                     [[NB, pl.npad - pl.b_base], [1, NB]])
                    if KSTAGE >= 4:
                        nc.gpsimd.dma_gather(
                            gbuf[:, :nsx, :], tbl_ap,
                            idx_sb[:, 8 * st0:8 * (st0 + nsx)],
                            P * nsx, P * nsx, NB, single_packet=False)
                    else:
                        nc.gpsimd.memset(gbuf[:, :nsx, :], 0.5)

                    nt = nsx // 4
                    for b0 in range(0, nt, 2):
                        nbt = min(2, nt - b0)
                        wb = 512 * nbt
                        ps1 = pps.tile([P, 1024], F32, tag="ps1")
                        e0 = (st0 + 4 * b0) * P
                        q, col = e0 // Q, e0 % Q
                        gt2 = sb.tile([NG, 1024], BF16, tag="gt2")
                        nc.sync.dma_start(gt2[:, :512 * nbt],
                                          g_dram[q, :, col:col + 512 * nbt])
                        for k in range(nbt):
                            nc.tensor.matmul(ps1[:, 512 * k:512 * (k + 1)],
                                             fw1b_sb[:, NB * i:NB * (i + 1)],
                                             gt2[:, 512 * k:512 * (k + 1)],
                                             start=True, stop=True)
                        nc.scalar.activation(ps1[:, :wb], ps1[:, :wb], AF.Exp,
                                             bias=fb1_sb[:, i:i + 1], scale=1.0)
                        x1 = sb.tile([P, 1024], BF16, tag="x1")
                        nc.scalar.activation(x1[:, :wb], ps1[:, :wb], AF.Ln,
                                             bias=1.0, scale=1.0)
                        for k in range(nbt):
                            tj = b0 + k
                            ps2 = pps.tile([P, 512], F32, tag="ps2")
                            nc.tensor.matmul(ps2[:], ones_sb[:],
                                             fb2e4_sb[0:1, 512 * i:512 * (i + 1)],
                                             start=True, stop=False,
                                             skip_group_check=True)
                            for s4 in range(4):
                                nc.tensor.matmul(
                                    ps2[:, P * s4:P * (s4 + 1)],
                                    x1[:, 512 * k + P * s4:512 * k + P * (s4 + 1)],
                                    fw2b_sb[:, NB * i:NB * (i + 1)],
                                    start=False, stop=True,
                                    skip_group_check=True)
                            msg = sb.tile([P, 512], BF16, tag="msg")
                            hfg = _ap(gbuf[:], 4 * tj * NB,
                                      [[pl.max_call_sub * NB, P], [1, 512]])
                            nc.vector.tensor_tensor(msg[:], ps2[:], hfg, op=OP.mult)
                            if k == 0:
                                stb = st0 + 4 * b0
                                nsb = 4 * nbt
                                Sm = spool.tile([P, 1024], BF16, tag="Sm")
                                if i == 0:
                                    dr_ap = _ap(dstrel_sb[:], stb,
                                                [[n_sub, P], [1, nsb], [0, P]])
                                    nc.vector.tensor_tensor(
                                        Sm[:, :P * nsb], iotaf_sb[:, :P * nsb],
                                        dr_ap, op=OP.is_equal)
                                    nc.sync.dma_start(
                                        s_dram[:, P * stb:P * (stb + nsb)],
                                        Sm[:, :P * nsb])
                                else:
                                    nc.sync.dma_start(
                                        Sm[:, :P * nsb],
                                        s_dram[:, P * stb:P * (stb + nsb)])
                            for s4 in range(4):
                                st = st0 + 4 * tj + s4
                                cki = int(pl.st_chunk[st])
                                if cki not in agg_open:
                                    agg_open[cki] = ppagg.tile(
                                        [P, P], F32, tag="aggps",
                                        name=f"aggps_{i}_{cki}")
                                first = (st == pl.chunk_first[cki])
                                last = (st == pl.chunk_last[cki])
                                nc.tensor.matmul(
                                    agg_open[cki][:],
                                    msg[:, P * s4:P * (s4 + 1)],
                                    Sm[:, P * (4 * k + s4):P * (4 * k + s4 + 1)],
                                    start=first, stop=last,
                                    skip_group_check=True)
                                if last:
                                    nc.vector.tensor_copy(
                                        agg_sb[:, P * cki:P * (cki + 1)],
                                        agg_open[cki][:])
                                    del agg_open[cki]

                # --- atom update --------------------------------------
                for c in range(K if KSTAGE >= 3 else 0):
                    ups = ppsm.tile([P, P], F32, tag="aggps", name=f"ups_{i}_{c}")
                    nc.tensor.matmul(ups[:], ow1_sb[:, NB * i:NB * (i + 1)],
                                     agg_sb[:, P * c:P * (c + 1)],
                                     start=True, stop=True)
                    ue = sb.tile([P, P], F32, tag="ue")
                    nc.scalar.activation(ue[:], ups[:], AF.Exp,
                                         bias=ob1_sb[:, i:i + 1], scale=1.0)
                    usb = sb.tile([P, P], F32, tag="usb")
                    nc.scalar.activation(usb[:], ue[:], AF.Ln,
                                         bias=1.0, scale=1.0)
                    drps = ppsm.tile([P, P], F32, tag="aggps", name=f"drps_{i}_{c}")
                    nc.tensor.matmul(drps[:], ow2_sb[:, NB * i:NB * (i + 1)],
                                     usb[:], start=True, stop=True)
                    drt = sb.tile([P, P], F32, tag="drt")
                    nc.vector.tensor_scalar(drt[:], drps[:],
                                            ob2e_sb[:, i:i + 1], None,
                                            op0=OP.add)
                    nc.vector.tensor_add(h_my[:, P * c:P * (c + 1)],
                                         h_my[:, P * c:P * (c + 1)], drt[:])

            # ================= readout =====================================
            e_acc = res.tile([1, NM], F32)
            nc.gpsimd.memset(e_acc[:], 0.0)
            for c in range(K):
                r1ps = ppsm.tile([NH, P], F32, tag="aggps", name=f"r1ps{c}")
                nc.tensor.matmul(r1ps[:], aw1_sb[:],
                                 h_my[:, P * c:P * (c + 1)],
                                 start=True, stop=True)
                r1e = sb.tile([NH, P], F32, tag="r1e")
                nc.scalar.activation(r1e[:], r1ps[:], AF.Exp,
                                     bias=ab1_sb[:, 0:1], scale=1.0)
                r1sb = sb.tile([NH, P], F32, tag="r1sb")
                nc.scalar.activation(r1sb[:], r1e[:], AF.Ln,
                                     bias=1.0, scale=1.0)
                yps = ppsm.tile([P, 1], F32, tag="aggps", name=f"yps{c}")
                nc.tensor.matmul(yps[:], r1sb[:], aw2_sb[:],
                                 start=True, stop=True)
                ysb = sb.tile([P, 1], F32, tag="ysb")
                nc.scalar.activation(ysb[:], yps[:], AF.Identity,
                                     bias=ab2p_sb[:, 0:1], scale=1.0)
                em_ps = ppsm.tile([1, NM], F32, tag="aggps", name=f"emps{c}")
                nc.tensor.matmul(em_ps[:], ysb[:],
                                 mask_sb[:, NM * c:NM * (c + 1)],
                                 start=True, stop=True)
                nc.vector.tensor_add(e_acc[:], e_acc[:], em_ps[:])
            nc.sync.dma_start(ypart[:], e_acc[:])

    # Spread gather descriptor-generation across the 4 SWDGE queues (Q7
    # core pairs), consistent with the DMASW semaphore lane Tile assigned
    # (the runtime locks each DMA semaphore to one SWDGE queue).
    import concourse.tile_sem_assignment as tsa
    sw_procs = {tsa.PROC_NAME_TO_IDX[f"DMASW{k}"]: k for k in range(8)}
    locked0 = set()
    gathers = []
    for b in nc.main_func.blocks:
        for inst in b.instructions:
            proc = getattr(inst, "bass_scheduled_proc", None)
            if proc in sw_procs:
                if isinstance(inst, mybir.InstDMAGatherAnt):
                    gathers.append((inst, sw_procs[proc]))
                else:
                    locked0.add(sw_procs[proc])
    for inst, lane in gathers:
        inst.queue_num = 0 if lane in locked0 else lane % 4

    nc.compile()
    return nc


# ----------------------------------------------------------------------------
# Entry point
# ----------------------------------------------------------------------------

_CACHE = {}


def _get_program(pl, NC, NM, coeff):
    key = (pl.n_atoms, pl.n_edges, pl.Ep, pl.K, NC, NM, round(coeff, 9))
    if key not in _CACHE:
        _CACHE[key] = build_program(pl, NC, NM, coeff)
    return _CACHE[key]


def kernel(r, xyz, a, n_per, embed, fw1, fb1, fw2, fb2, afw, afb,
           ow1, ob1, ow2, ob2, aw1, ab1, aw2, ab2, trace=False):
    r = np.asarray(r)
    xyz = np.asarray(xyz, dtype=np.float32)
    a = np.asarray(a)
    weights = dict(fw1=np.asarray(fw1), fb1=np.asarray(fb1),
                   fw2=np.asarray(fw2), fb2=np.asarray(fb2),
                   afw=np.asarray(afw), afb=np.asarray(afb),
                   ow1=np.asarray(ow1), ob1=np.asarray(ob1),
                   ow2=np.asarray(ow2), ob2=np.asarray(ob2),
                   aw1=np.asarray(aw1), ab1=np.asarray(ab1),
                   aw2=np.asarray(aw2), ab2=np.asarray(ab2))
    pl = make_plan(r, xyz, a, int(n_per), n_cores=8)
    in_maps, coeff = make_inputs(pl, r, xyz, a, np.asarray(embed), weights)
    NC = weights["fw1"].shape[0]
    nc = _get_program(pl, NC, pl.n_mol, coeff)
    res = bass_utils.run_bass_kernel_spmd(
        nc, in_maps, core_ids=list(range(pl.n_cores)), trace=trace)
    out = np.zeros(pl.n_mol, dtype=np.float64)
    for k in range(pl.n_cores):
        out += res.results[k]["ypart"][0].astype(np.float64)
    kernel._last_results = res
    return out.astype(np.float32)



# revision 7
# speedup vs baseline: 1.7473x; 1.7473x over previous
"""SchNet-style GNN message passing on 8 Trainium2 NeuronCores.

Strategy (per sharding hint): edges sharded by destination atom across 8
cores; atoms relabeled + degree-balanced so each core owns an equal shard
of destination atoms, with edges padded into a fixed, SPMD-uniform static
schedule.  Small weights replicated.  Per conv: each core computes the
atom-filter features hf = h @ afw for ITS OWN atom shard in row-major
[atoms, NB] bf16 layout, and one AllGather concatenates the shards into
the full gather table (shared scratchpad) read by hardware gather-DMA;
the scatter-add (segment sum) is done on the tensor engine as one-hot
matmuls into PSUM accumulators (edges pre-sorted by destination chunk).
Gather index streams are padded with trailing -1 entries which the SWDGE
descriptor generator skips.  Final per-molecule energies come from a mask
matmul; host sums the 8 partial [n_mol] vectors.
"""

import os
import sys
import numpy as np

sys.path.insert(0, "/opt/trn_rl_repo")

from contextlib import ExitStack

import ml_dtypes
import concourse.bass as bass
import concourse.tile as tile
import concourse.bacc as bacc
from concourse import mybir
from concourse import bass_utils

F32 = mybir.dt.float32
BF16 = mybir.dt.bfloat16
I16 = mybir.dt.int16
AF = mybir.ActivationFunctionType
OP = mybir.AluOpType

LN2 = float(np.log(2.0))
EPS = 1e-12
P = 128          # partitions / chunk size
NG = 32          # gaussians
NB = 128         # atom basis / filters
NH = 64          # readout hidden

USE_SOFTPLUS = int(os.environ.get("USE_SOFTPLUS", "1"))
NEG_IDX = int(os.environ.get("NEG_IDX", "1"))
PREFILL = int(os.environ.get("PREFILL", "1"))
SHARED_AG = int(os.environ.get("SHARED_AG", "1"))


# ----------------------------------------------------------------------------
# Host-side plan: atom relabeling, edge sharding, static schedule
# ----------------------------------------------------------------------------

class Plan:
    pass


def _greedy_pack(deg_a, deg_b, atom_ids, n_bins, rng):
    """Pack len(atom_ids) atoms into n_bins bins of exactly P atoms each,
    balancing per-bin sums of deg_a and deg_b.  Returns [n_bins, P] atom ids
    (-1 for none -> caller guarantees exact fit)."""
    n = len(atom_ids)
    assert n == n_bins * P
    tot_a = max(float(deg_a[atom_ids].sum()), 1.0)
    tot_b = max(float(deg_b[atom_ids].sum()), 1.0)
    ta = tot_a / n_bins
    tb = tot_b / n_bins
    order = np.argsort(-(deg_a[atom_ids] + deg_b[atom_ids]), kind="stable")
    sa = np.zeros(n_bins)
    sb = np.zeros(n_bins)
    cnt = np.zeros(n_bins, dtype=np.int64)
    bins = np.full((n_bins, P), -1, dtype=np.int64)
    for oi in order:
        a = atom_ids[oi]
        da, db = deg_a[a], deg_b[a]
        load = np.maximum((sa + da) / ta, (sb + db) / tb)
        load[cnt >= P] = np.inf
        i = int(np.argmin(load))
        bins[i, cnt[i]] = a
        cnt[i] += 1
        sa[i] += da
        sb[i] += db
    assert (cnt == P).all()
    return bins, sa, sb


def make_plan(r, xyz, a, n_per, n_cores=8):
    pl = Plan()
    n_atoms = xyz.shape[0]
    n_edges = a.shape[0]
    rng = np.random.default_rng(12345)

    # padded atom count: multiple of n_cores*P
    npad = ((n_atoms + n_cores * P - 1) // (n_cores * P)) * (n_cores * P)
    K = npad // (n_cores * P)          # bins (chunks) per core
    SH = K * P                          # atoms per core shard
    a_cap = min(32768, npad)            # atoms addressable by table A (idx16)
    a_cap = (a_cap // P) * P
    if a_cap < npad // 2:
        raise ValueError("atom table too large for two-base gather split")
    b_base = npad - a_cap               # B gather base row

    dst = a[:, 0].astype(np.int64)
    src = a[:, 1].astype(np.int64)

    # choose A-set (atoms whose NEW id < a_cap): random choice of a_cap reals;
    # virtual atoms (degree 0) fill whatever space remains in each group.
    n_virt = npad - n_atoms
    n_aset = min(a_cap, n_atoms)
    perm_r = rng.permutation(n_atoms)
    aset = np.zeros(n_atoms, dtype=bool)
    aset[perm_r[:n_aset]] = True

    in_a = aset[src]                    # edge half by src membership
    degA = np.bincount(dst[in_a], minlength=n_atoms)
    degB = np.bincount(dst[~in_a], minlength=n_atoms)
    degA_x = np.concatenate([degA, np.zeros(n_virt, dtype=degA.dtype)])
    degB_x = np.concatenate([degB, np.zeros(n_virt, dtype=degB.dtype)])

    virt_ids = np.arange(n_atoms, npad)
    n_virt_a = a_cap - n_aset            # virtuals needed in the A group
    a_ids = np.concatenate([np.nonzero(aset)[0], virt_ids[:n_virt_a]])
    b_ids_x = np.concatenate([np.nonzero(~aset)[0], virt_ids[n_virt_a:]])
    binsA, saA, sbA = _greedy_pack(degA_x, degB_x, a_ids, a_cap // P, rng)
    if npad > a_cap:
        binsB, saB, sbB = _greedy_pack(degA_x, degB_x, b_ids_x,
                                       (npad - a_cap) // P, rng)
    else:
        binsB = np.zeros((0, P), dtype=np.int64)
        saB = np.zeros(1)
        sbB = np.zeros(1)

    new_of_old = np.full(npad, -1, dtype=np.int64)
    allbins = np.concatenate([binsA, binsB], axis=0)    # [npad//P, P]
    flat = allbins.reshape(-1)
    new_of_old[flat] = np.arange(npad)
    old_of_new = flat                                    # new id -> old id

    maxA = int(np.maximum(saA.max(), saB.max()))
    maxB = int(np.maximum(sbA.max(), sbB.max()))
    TA = ((maxA + P - 1) // P + 3) // 4 * 4              # subtiles, mult of 4
    TA = max(TA, 4)
    TB = ((maxB + P - 1) // P + 1) // 2 * 2              # mult of 2
    TB = max(TB, 2)

    # static stream structure (identical for every core)
    groups = [(c, c + 1) for c in range(0, K - 1, 2)]
    if K % 2 == 1:
        groups.append((K - 1,))
    n_sub_main = sum(len(g) * (TA + TB) for g in groups)
    padb = (16 - (n_sub_main % 16)) % 16                # pad to 2048-edge mult
    # also need last B span % 4 == 0
    lastB = len(groups[-1]) * TB + padb
    while lastB % 4 != 0:
        padb += 16
        lastB = len(groups[-1]) * TB + padb
    n_sub = n_sub_main + padb
    Ep = n_sub * P

    # per-subtile chunk binding + gather call table (same for all cores).
    # A spans get one call per chunk, so per-chunk padding is trailing and
    # the SWDGE generator can skip it (negative idxs); B spans get one call
    # per group, split into <= CALLSUB pieces (multiples of 4 subtiles).
    CALLSUB = min(32, max(TA, 2 * TB))
    CALLSUB = (CALLSUB // 4) * 4
    st_chunk = np.zeros(n_sub, dtype=np.int64)
    calls = []            # (start_subtile, n_subtiles, half)  half: 0=A 1=B
    s = 0
    span_start = {}
    for gi, g in enumerate(groups):
        for c in g:
            span_start[(c, 0)] = s
            st_chunk[s:s + TA] = c
            off = 0
            while off < TA:
                take = min(CALLSUB, TA - off)
                calls.append((s + off, take, 0))
                off += take
            s += TA
        b0 = s
        for c in g:
            span_start[(c, 1)] = s
            st_chunk[s:s + TB] = c
            s += TB
        if gi == len(groups) - 1 and padb:
            st_chunk[s:s + padb] = g[-1]
            s += padb
        blen = s - b0
        off = 0
        while off < blen:
            take = min(CALLSUB, blen - off)
            calls.append((b0 + off, take, 1))
            off += take
    assert s == n_sub
    max_call_sub = max(ns for _, ns, _ in calls)
    assert all(ns % 4 == 0 for _, ns, _ in calls)

    # last subtile index of each chunk (for psum close / stop flag)
    chunk_last = np.zeros(K, dtype=np.int64)
    chunk_first = np.zeros(K, dtype=np.int64)
    seen = set()
    for st in range(n_sub):
        c = st_chunk[st]
        if c not in seen:
            chunk_first[c] = st
            seen.add(c)
        chunk_last[c] = st

    # ---- per-core edge data -------------------------------------------------
    src_new = new_of_old[src]
    dst_new = new_of_old[dst]
    e_core = dst_new // SH
    e_chunk = (dst_new % SH) // P
    e_half = (src_new >= a_cap).astype(np.int64)

    idx_lin = np.full((n_cores, Ep), -1 if NEG_IDX else 0, dtype=np.int16)
    dstrel_lin = np.full((n_cores, Ep), -1.0, dtype=np.float32)
    osrc_lin = np.zeros((n_cores, Ep), dtype=np.int64)
    odst_lin = np.zeros((n_cores, Ep), dtype=np.int64)

    # bucket edges by (core, chunk, half)
    order = np.lexsort((e_half, e_chunk, e_core))
    so_src, so_dst = src_new[order], dst_new[order]
    so_core, so_chunk, so_half = e_core[order], e_chunk[order], e_half[order]
    so_osrc, so_odst = src[order], dst[order]
    # boundaries
    keys = so_core * (K * 2) + so_chunk * 2 + so_half
    bstart = np.searchsorted(keys, np.arange(n_cores * K * 2), side="left")
    bend = np.searchsorted(keys, np.arange(n_cores * K * 2), side="right")

    for core in range(n_cores):
        for c in range(K):
            for h in (0, 1):
                bi = core * (K * 2) + c * 2 + h
                e0, e1 = bstart[bi], bend[bi]
                cnt = e1 - e0
                cap = (TA if h == 0 else TB) * P
                assert cnt <= cap, (core, c, h, cnt, cap)
                p0 = span_start[(c, h)] * P
                sl = slice(p0, p0 + cnt)
                if h == 0:
                    idx_lin[core, sl] = so_src[e0:e1].astype(np.int16)
                else:
                    idx_lin[core, sl] = (so_src[e0:e1] - b_base).astype(np.int16)
                dstrel_lin[core, sl] = (so_dst[e0:e1] % P).astype(np.float32)
                osrc_lin[core, sl] = so_osrc[e0:e1]
                odst_lin[core, sl] = so_odst[e0:e1]

    # interior padding (pad rows followed by a real row within the same
    # call) must gather a safe row (0); only trailing pads stay -1 so the
    # SWDGE descriptor generator drops them.
    # Per-call gather count, uniform across cores (SPMD): the value-trim in
    # the SWDGE ucode and the ring-space reservation in the decode stage must
    # agree, and the decode uses num_idxs_reg.  cnt = max real rows over
    # cores (16-rounded); pads below cnt gather row 0, rows >= cnt are -1 and
    # are skipped by every core identically.
    call_cnt = []
    for (st0, nsx, half) in calls:
        r0, r1 = st0 * P, (st0 + nsx) * P
        cnt = 0
        for core in range(n_cores):
            real = np.nonzero(dstrel_lin[core, r0:r1] >= 0)[0]
            if len(real):
                cnt = max(cnt, int(real[-1]) + 1)
        cnt = min((cnt + 15) // 16 * 16, nsx * P)
        call_cnt.append(cnt)
        if NEG_IDX:
            for core in range(n_cores):
                seg = idx_lin[core, r0:r0 + cnt]
                seg[dstrel_lin[core, r0:r0 + cnt] < 0] = 0
            idx_lin[:, r0 + cnt:r1] = -1
        else:
            call_cnt[-1] = nsx * P
            for core in range(n_cores):
                seg = idx_lin[core, r0:r1]
                seg[dstrel_lin[core, r0:r1] < 0] = 0

    pl.n_atoms, pl.n_edges, pl.npad = n_atoms, n_edges, npad
    pl.n_cores, pl.K, pl.SH, pl.Ep, pl.n_sub = n_cores, K, SH, Ep, n_sub
    pl.TA, pl.TB, pl.padb = TA, TB, padb
    pl.a_cap, pl.b_base = a_cap, b_base
    pl.groups, pl.calls, pl.max_call_sub = groups, calls, max_call_sub
    pl.call_cnt = call_cnt
    pl.st_chunk, pl.chunk_first, pl.chunk_last = st_chunk, chunk_first, chunk_last
    pl.new_of_old, pl.old_of_new = new_of_old, old_of_new
    pl.idx_lin, pl.dstrel_lin = idx_lin, dstrel_lin
    pl.osrc_lin, pl.odst_lin = osrc_lin, odst_lin
    pl.n_per = int(n_per)
    pl.n_mol = n_atoms // pl.n_per
    return pl


def make_inputs(pl, r, xyz, a, embed, weights):
    """Build per-core in_maps.  weights: dict of raw weight arrays."""
    C, K, SH, Ep, n_sub = pl.n_cores, pl.K, pl.SH, pl.Ep, pl.n_sub
    NC = weights["fw1"].shape[0]
    NM = pl.n_mol
    F0 = Ep // P

    h0_all = embed[r[:, 0].astype(np.int64)].astype(np.float32)     # [n,NB]
    h0_new = np.zeros((pl.npad, NB), dtype=np.float32)
    real = pl.old_of_new < pl.n_atoms
    h0_new[real] = h0_all[pl.old_of_new[real]]

    mol_new = np.full(pl.npad, -1, dtype=np.int64)
    mol_new[real] = pl.old_of_new[real] // pl.n_per

    xyzf = xyz.astype(np.float32)

    fw1, fb1 = weights["fw1"], weights["fb1"]
    fw2, fb2 = weights["fw2"], weights["fb2"]
    afw, afb = weights["afw"], weights["afb"]
    ow1, ob1 = weights["ow1"], weights["ob1"]
    ow2, ob2 = weights["ow2"], weights["ob2"]
    aw1, ab1 = weights["aw1"], weights["ab1"]
    aw2, ab2 = weights["aw2"], weights["ab2"]
    assert np.all(afb == 0.0), "nonzero afb not supported by this kernel"

    # fold ssp's -log(2) into the following layer's bias
    fb2e = (fb2 - LN2 * fw2.sum(axis=1)).astype(np.float32)         # [NC,NB]
    ob2e = (ob2 - LN2 * ow2.sum(axis=1)).astype(np.float32)         # [NC,NB]
    ab2e = float(ab2[0] - LN2 * aw2.sum(axis=0)[0])

    offs = np.linspace(0.0, 5.0, NG).astype(np.float32)
    width = float(offs[1] - offs[0])
    coeff = -0.5 / (width * width)

    # fb2 replicated across partitions: [P, NC*4*NB]
    fb2rep = np.concatenate([np.tile(fb2e[i], (P, 4)) for i in range(NC)],
                            axis=1).astype(np.float32)

    shared = {
        "fw1b": np.ascontiguousarray(
            fw1.transpose(1, 0, 2).reshape(NG, NC * NB)).astype(ml_dtypes.bfloat16),
        "fw2b": np.ascontiguousarray(
            fw2.transpose(1, 0, 2).reshape(NB, NC * NB)).astype(ml_dtypes.bfloat16),
        "afwb": np.ascontiguousarray(
            afw.transpose(1, 0, 2).reshape(NB, NC * NB)).astype(ml_dtypes.bfloat16),
        "ow1w": np.ascontiguousarray(
            ow1.transpose(1, 0, 2).reshape(NB, NC * NB)).astype(np.float32),
        "ow2w": np.ascontiguousarray(
            ow2.transpose(1, 0, 2).reshape(NB, NC * NB)).astype(np.float32),
        "aw1w": aw1.astype(np.float32),                              # [NB,NH]
        "aw2w": aw2.astype(np.float32),                              # [NH,1]
        "fb1t": np.ascontiguousarray(fb1.T).astype(np.float32),      # [NB,NC]
        "ob1t": np.ascontiguousarray(ob1.T).astype(np.float32),
        "ob2et": np.ascontiguousarray(ob2e.T).astype(np.float32),
        "fb2rep": fb2rep,                                            # [P,NC*512]
        "ab1t": ab1.reshape(NH, 1).astype(np.float32),
        "ab2p": np.full((P, 1), ab2e, dtype=np.float32),
        "negmu": np.tile(-offs, 4).reshape(P, 1).astype(np.float32),
        "iota512": np.tile(np.arange(P, dtype=np.float32), (P, 4)),
    }

    in_maps = []
    for c in range(C):
        m = dict(shared)
        osrc = pl.osrc_lin[c]
        odst = pl.odst_lin[c]
        xs = xyzf[osrc]          # [Ep,3]
        xd = xyzf[odst]
        m["xsx"] = np.ascontiguousarray(xs[:, 0].reshape(P, F0))
        m["xsy"] = np.ascontiguousarray(xs[:, 1].reshape(P, F0))
        m["xsz"] = np.ascontiguousarray(xs[:, 2].reshape(P, F0))
        m["xdx"] = np.ascontiguousarray(xd[:, 0].reshape(P, F0))
        m["xdy"] = np.ascontiguousarray(xd[:, 1].reshape(P, F0))
        m["xdz"] = np.ascontiguousarray(xd[:, 2].reshape(P, F0))
        m["idx"] = np.ascontiguousarray(
            np.tile(pl.idx_lin[c].reshape(Ep // 16, 16).T, (8, 1)))
        m["dstrel"] = np.ascontiguousarray(
            pl.dstrel_lin[c].reshape(n_sub, P).T)
        m["h0t"] = np.ascontiguousarray(
            h0_new[c * SH:(c + 1) * SH].T)                          # [NB,SH]
        msk = np.zeros((K, P, NM), dtype=np.float32)
        mols = mol_new[c * SH:(c + 1) * SH].reshape(K, P)
        for mm in range(NM):
            msk[:, :, mm] = (mols == mm)
        m["mask"] = msk
        in_maps.append(m)
    return in_maps, coeff


# ----------------------------------------------------------------------------
# Device program
# ----------------------------------------------------------------------------

def _ap(tile_ap, extra_off, pattern):
    """Raw access-pattern surgery on a (pool-tile or dram) AP."""
    return bass.AP(tile_ap.tensor, tile_ap.offset + extra_off, pattern)


def _patch_act_tables():
    """Pin each activation function to exactly one ACT table so bacc never
    thrashes table loads: Softplus/Copy/Identity -> softplus_and_others
    (Softplus is missing from act_info's listing but present in the HW
    table), Exp -> exp_and_others, Sqrt -> sqrt_and_others."""
    if getattr(bacc, "_act_tables_patched", False):
        return
    orig = bacc.get_activation_tables

    if USE_SOFTPLUS:
        def patched(arch):
            t = dict(orig(arch))
            shared = {AF.Identity, AF.Copy, AF.Square}
            for name in list(t):
                s = set(t[name])
                if name == "softplus_and_others":
                    s |= {AF.Softplus}
                else:
                    s -= shared | {AF.Softplus}
                if name != "exp_and_others":
                    s -= {AF.Exp}
                if name != "sqrt_and_others":
                    s -= {AF.Sqrt}
                t[name] = s
            return t
    else:
        def patched(arch):
            t = dict(orig(arch))
            shared = {AF.Exp, AF.Ln, AF.Identity, AF.Copy, AF.Square}
            for name in list(t):
                if name != "natural_log_exp_and_others":
                    t[name] = t[name] - shared
            return t

    bacc.get_activation_tables = patched
    bacc._act_tables_patched = True


def build_program(pl, NC, NM, coeff):
    _patch_act_tables()
    C, K, SH, Ep, n_sub = pl.n_cores, pl.K, pl.SH, pl.Ep, pl.n_sub
    F0 = Ep // P
    Q = Ep // 4                      # edges per gaussian partition-group
    NW = 4                           # phase-0 g-build col iterations
    while Q % NW != 0 or (Q // NW) > 1024:
        NW *= 2
    Wg = Q // NW
    CS = pl.max_call_sub

    nc = bacc.Bacc("TRN2", target_bir_lowering=False, debug=False,
                   enable_asserts=False, num_devices=C, num_swdge_queues=4)

    def din(name, shape, dt=F32):
        return nc.dram_tensor(name, shape, dt, kind="ExternalInput").ap()

    xsx, xsy, xsz = din("xsx", [P, F0]), din("xsy", [P, F0]), din("xsz", [P, F0])
    xdx, xdy, xdz = din("xdx", [P, F0]), din("xdy", [P, F0]), din("xdz", [P, F0])
    idx_d = din("idx", [P, Ep // 16], I16)
    dstrel_d = din("dstrel", [P, n_sub])
    h0t_d = din("h0t", [NB, SH])
    mask_d = din("mask", [K, P, NM])
    fw1b_d = din("fw1b", [NG, NC * NB], BF16)
    fw2b_d = din("fw2b", [NB, NC * NB], BF16)
    afwb_d = din("afwb", [NB, NC * NB], BF16)
    ow1w_d = din("ow1w", [NB, NC * NB])
    ow2w_d = din("ow2w", [NB, NC * NB])
    aw1w_d = din("aw1w", [NB, NH])
    aw2w_d = din("aw2w", [NH, 1])
    fb1t_d = din("fb1t", [NB, NC])
    ob1t_d = din("ob1t", [NB, NC])
    ob2et_d = din("ob2et", [NB, NC])
    fb2rep_d = din("fb2rep", [P, NC * 4 * NB])
    ab1t_d = din("ab1t", [NH, 1])
    ab2p_d = din("ab2p", [P, 1])
    negmu_d = din("negmu", [P, 1])
    iota512_d = din("iota512", [P, 4 * P])

    ypart = nc.dram_tensor("ypart", [1, NM], F32, kind="ExternalOutput").ap()

    with tile.TileContext(nc) as tc:
        with ExitStack() as ctx:
            dram = ctx.enter_context(tc.tile_pool(name="dram", bufs=1, space="DRAM"))
            res = ctx.enter_context(tc.tile_pool(name="res", bufs=1))
            sb = ctx.enter_context(tc.tile_pool(name="sb", bufs=3))
            gpool = ctx.enter_context(tc.tile_pool(name="gpool", bufs=3))
            p0 = ctx.enter_context(tc.tile_pool(name="p0", bufs=2))
            spool = ctx.enter_context(tc.tile_pool(name="spool", bufs=3))
            pps = ctx.enter_context(tc.tile_pool(name="pps", bufs=2, space="PSUM"))
            ppagg = ctx.enter_context(tc.tile_pool(name="ppagg", bufs=2, space="PSUM"))
            ppu = ctx.enter_context(tc.tile_pool(name="ppu", bufs=2, space="PSUM"))

            # ---- DRAM scratch ----
            d_dram = dram.tile([P, F0], F32)
            g_dram = dram.tile([4, NG, Q], BF16)
            ag_space = "Shared" if SHARED_AG else "Local"
            hf_my = [dram.tile([SH, NB], BF16, name=f"hf_my{i}")
                     for i in range(NC)]
            hf_tab = [dram.tile([pl.npad, NB], BF16, addr_space=ag_space,
                                name=f"hf_tab{i}")
                      for i in range(NC)]

            # ---- resident SBUF ----
            h_my = res.tile([NB, SH], F32)
            agg_sb = res.tile([NB, SH], F32)
            idx_sb = res.tile([P, Ep // 16], I16)
            dstrel_sb = res.tile([P, n_sub], F32)
            iota_sb = res.tile([P, 4 * P], F32)
            mask_sb = res.tile([P, K * NM], F32)
            fw1b_sb = res.tile([NG, NC * NB], BF16)
            fw2b_sb = res.tile([NB, NC * NB], BF16)
            afwb_sb = res.tile([NB, NC * NB], BF16)
            ow1_sb = res.tile([NB, NC * NB], F32)
            ow2_sb = res.tile([NB, NC * NB], F32)
            aw1_sb = res.tile([NB, NH], F32)
            aw2_sb = res.tile([NH, 1], F32)
            fb1_sb = res.tile([NB, NC], F32)
            ob1_sb = res.tile([NB, NC], F32)
            ob2e_sb = res.tile([NB, NC], F32)
            fb2rep_sb = res.tile([P, NC * 4 * NB], F32)
            ab1_sb = res.tile([NH, 1], F32)
            ab2p_sb = res.tile([P, 1], F32)
            negmu_sb = res.tile([P, 1], F32)
            epsb_sb = res.tile([P, 1], F32)
            e_acc = res.tile([1, NM], F32)
            nc.vector.memset(epsb_sb[:], EPS)
            nc.vector.memset(e_acc[:], 0.0)

            nc.sync.dma_start(h_my[:], h0t_d[:])
            nc.sync.dma_start(idx_sb[:], idx_d[:])
            nc.sync.dma_start(dstrel_sb[:], dstrel_d[:])
            nc.sync.dma_start(iota_sb[:], iota512_d[:])
            # mask [K,P,NM] -> [P, K*NM]
            nc.sync.dma_start(
                _ap(mask_sb[:], 0, [[K * NM, P], [NM, K], [1, NM]]),
                _ap(mask_d, 0, [[NM, P], [P * NM, K], [1, NM]]))
            for t_sb, t_d in [(fw1b_sb, fw1b_d), (fw2b_sb, fw2b_d),
                              (afwb_sb, afwb_d), (ow1_sb, ow1w_d),
                              (ow2_sb, ow2w_d), (aw1_sb, aw1w_d),
                              (aw2_sb, aw2w_d), (fb1_sb, fb1t_d),
                              (ob1_sb, ob1t_d), (ob2e_sb, ob2et_d),
                              (fb2rep_sb, fb2rep_d), (ab1_sb, ab1t_d),
                              (ab2p_sb, ab2p_d), (negmu_sb, negmu_d)]:
                nc.sync.dma_start(t_sb[:], t_d[:])

            # zero-init gather buffers once so skipped (trailing-pad) rows
            # always hold finite values
            for z in range(3):
                gz = gpool.tile([P, CS * NB], BF16, tag="gbuf", name=f"gz{z}")
                nc.vector.memset(gz[:], 0.0)

            def emit_hf_chunk(i, c):
                """hf rows for chunk c of conv i from current h_my."""
                hb = sb.tile([NB, P], BF16, tag="hb")
                nc.vector.tensor_copy(hb[:], h_my[:, P * c:P * (c + 1)])
                hfps = ppu.tile([P, P], F32, tag="upd", name=f"hfps_{i}_{c}")
                nc.tensor.matmul(hfps[:], hb[:],
                                 afwb_sb[:, NB * i:NB * (i + 1)],
                                 start=True, stop=True)
                hfsb = sb.tile([P, P], BF16, tag="hfsb")
                nc.scalar.copy(hfsb[:], hfps[:])
                nc.sync.dma_start(hf_my[i][P * c:P * (c + 1), :], hfsb[:])

            def emit_update_chunk(i, c):
                """h += dense(ssp(dense(agg)))  for chunk c, conv i."""
                ups = ppu.tile([P, P], F32, tag="upd", name=f"ups_{i}_{c}")
                nc.tensor.matmul(ups[:], ow1_sb[:, NB * i:NB * (i + 1)],
                                 agg_sb[:, P * c:P * (c + 1)],
                                 start=True, stop=True)
                usb = sb.tile([P, P], F32, tag="usb")
                if USE_SOFTPLUS:
                    nc.scalar.activation(usb[:], ups[:], AF.Softplus,
                                         bias=ob1_sb[:, i:i + 1], scale=1.0)
                else:
                    ue = sb.tile([P, P], F32, tag="ue")
                    nc.scalar.activation(ue[:], ups[:], AF.Exp,
                                         bias=ob1_sb[:, i:i + 1], scale=1.0)
                    nc.scalar.activation(usb[:], ue[:], AF.Ln,
                                         bias=1.0, scale=1.0)
                drps = ppu.tile([P, P], F32, tag="upd", name=f"drps_{i}_{c}")
                nc.tensor.matmul(drps[:], ow2_sb[:, NB * i:NB * (i + 1)],
                                 usb[:], start=True, stop=True)
                drt = sb.tile([P, P], F32, tag="drt")
                nc.vector.tensor_scalar(drt[:], drps[:],
                                        ob2e_sb[:, i:i + 1], None,
                                        op0=OP.add)
                nc.vector.tensor_add(h_my[:, P * c:P * (c + 1)],
                                     h_my[:, P * c:P * (c + 1)], drt[:])

            def emit_readout_chunk(c):
                r1ps = ppu.tile([NH, P], F32, tag="upd", name=f"r1ps{c}")
                nc.tensor.matmul(r1ps[:], aw1_sb[:],
                                 h_my[:, P * c:P * (c + 1)],
                                 start=True, stop=True)
                r1sb = sb.tile([NH, P], F32, tag="r1sb")
                if USE_SOFTPLUS:
                    nc.scalar.activation(r1sb[:], r1ps[:], AF.Softplus,
                                         bias=ab1_sb[:, 0:1], scale=1.0)
                else:
                    r1e = sb.tile([NH, P], F32, tag="r1e")
                    nc.scalar.activation(r1e[:], r1ps[:], AF.Exp,
                                         bias=ab1_sb[:, 0:1], scale=1.0)
                    nc.scalar.activation(r1sb[:], r1e[:], AF.Ln,
                                         bias=1.0, scale=1.0)
                yps = ppu.tile([P, 1], F32, tag="upd", name=f"yps{c}")
                nc.tensor.matmul(yps[:], r1sb[:], aw2_sb[:],
                                 start=True, stop=True)
                ysb = sb.tile([P, 1], F32, tag="ysb")
                nc.scalar.activation(ysb[:], yps[:], AF.Identity,
                                     bias=ab2p_sb[:, 0:1], scale=1.0)
                em_ps = ppu.tile([1, NM], F32, tag="upd", name=f"emps{c}")
                nc.tensor.matmul(em_ps[:], ysb[:],
                                 mask_sb[:, NM * c:NM * (c + 1)],
                                 start=True, stop=True)
                nc.vector.tensor_add(e_acc[:], e_acc[:], em_ps[:])

            # startup: hf table for conv 0 from h0
            for c in range(K):
                emit_hf_chunk(0, c)

            # ================= phase 0: distances and gaussians ============
            cx = p0.tile([P, F0], F32, tag="ph0", bufs=1)
            cy = p0.tile([P, F0], F32, tag="ph0b", bufs=1)
            cz = p0.tile([P, F0], F32, tag="ph0c", bufs=1)
            tx = p0.tile([P, F0], F32, tag="ph0d", bufs=1)
            nc.sync.dma_start(cx[:], xsx[:])
            nc.sync.dma_start(tx[:], xdx[:])
            nc.vector.tensor_sub(cx[:], cx[:], tx[:])
            nc.vector.tensor_mul(cx[:], cx[:], cx[:])
            nc.sync.dma_start(cy[:], xsy[:])
            nc.sync.dma_start(tx[:], xdy[:])
            nc.vector.tensor_sub(cy[:], cy[:], tx[:])
            nc.vector.tensor_mul(cy[:], cy[:], cy[:])
            nc.sync.dma_start(cz[:], xsz[:])
            nc.sync.dma_start(tx[:], xdz[:])
            nc.vector.tensor_sub(cz[:], cz[:], tx[:])
            nc.vector.tensor_mul(cz[:], cz[:], cz[:])
            nc.vector.tensor_add(cx[:], cx[:], cy[:])
            nc.vector.tensor_add(cx[:], cx[:], cz[:])
            nc.scalar.activation(cy[:], cx[:], AF.Sqrt,
                                 bias=epsb_sb[:, 0:1], scale=1.0)
            nc.sync.dma_start(d_dram[:], cy[:])

            for w in range(NW):
                dbc = p0.tile([P, Wg], F32, tag="dbc")
                nc.sync.dma_start(
                    dbc[:], _ap(d_dram[:], w * Wg, [[Q, 4], [0, NG], [1, Wg]]))
                t1 = p0.tile([P, Wg], F32, tag="t1")
                nc.vector.tensor_scalar(t1[:], dbc[:], negmu_sb[:, 0:1], None,
                                        op0=OP.add)
                nc.vector.tensor_mul(t1[:], t1[:], t1[:])
                gt = p0.tile([P, Wg], BF16, tag="gt0")
                nc.scalar.activation(gt[:], t1[:], AF.Exp, bias=0.0, scale=coeff)
                nc.sync.dma_start(
                    _ap(g_dram[:], w * Wg, [[NG * Q, 4], [Q, NG], [1, Wg]]),
                    gt[:])

            # ================= conv layers =================================
            for i in range(NC):
                # --- allgather hf shards into the full table ------------
                nc.gpsimd.collective_compute(
                    "AllGather", OP.bypass,
                    replica_groups=[list(range(C))],
                    ins=[hf_my[i].opt()], outs=[hf_tab[i].opt()])

                # --- edge phase ---------------------------------------
                agg_open = {}
                for ci, (st0, nsx, half) in enumerate(pl.calls):
                    gbuf = gpool.tile([P, CS * NB], BF16, tag="gbuf")
                    if half == 0:
                        tbl_ap = _ap(hf_tab[i][:], 0,
                                     [[NB, pl.a_cap], [1, NB]])
                    else:
                        tbl_ap = _ap(hf_tab[i][:], pl.b_base * NB,
                                     [[NB, pl.npad - pl.b_base], [1, NB]])
                    nc.gpsimd.dma_gather(
                        _ap(gbuf[:], 0, [[CS * NB, P], [NB, nsx], [1, NB]]),
                        tbl_ap,
                        idx_sb[:, 8 * st0:8 * (st0 + nsx)],
                        P * nsx, pl.call_cnt[ci], NB, single_packet=False)

                    for b in range(nsx // 4):
                        stb = st0 + 4 * b
                        e0 = stb * P
                        q, col = e0 // Q, e0 % Q
                        gt2 = sb.tile([NG, 512], BF16, tag="gt2")
                        nc.sync.dma_start(gt2[:], g_dram[q, :, col:col + 512])
                        ps1 = pps.tile([P, 512], F32, tag="ps1")
                        nc.tensor.matmul(ps1[:],
                                         fw1b_sb[:, NB * i:NB * (i + 1)],
                                         gt2[:], start=True, stop=True)
                        x1 = sb.tile([P, 512], BF16, tag="x1")
                        if USE_SOFTPLUS:
                            nc.scalar.activation(x1[:], ps1[:], AF.Softplus,
                                                 bias=fb1_sb[:, i:i + 1],
                                                 scale=1.0)
                        else:
                            nc.scalar.activation(ps1[:], ps1[:], AF.Exp,
                                                 bias=fb1_sb[:, i:i + 1],
                                                 scale=1.0)
                            nc.scalar.activation(x1[:], ps1[:], AF.Ln,
                                                 bias=1.0, scale=1.0)
                        ps2 = pps.tile([P, 512], F32, tag="ps2")
                        if PREFILL:
                            nc.scalar.copy(
                                ps2[:], fb2rep_sb[:, 512 * i:512 * (i + 1)])
                        for s4 in range(4):
                            nc.tensor.matmul(
                                ps2[:, P * s4:P * (s4 + 1)],
                                x1[:, P * s4:P * (s4 + 1)],
                                fw2b_sb[:, NB * i:NB * (i + 1)],
                                start=not PREFILL, stop=True,
                                skip_group_check=True)
                        Sm = spool.tile([P, 512], BF16, tag="Sm")
                        dr_ap = _ap(dstrel_sb[:], stb,
                                    [[n_sub, P], [1, 4], [0, P]])
                        nc.vector.tensor_tensor(Sm[:], iota_sb[:], dr_ap,
                                                op=OP.is_equal)
                        msg = sb.tile([P, 512], BF16, tag="msg")
                        if PREFILL:
                            hfg = _ap(gbuf[:], 4 * b * NB,
                                      [[CS * NB, P], [1, 512]])
                            nc.vector.tensor_tensor(msg[:], ps2[:], hfg,
                                                    op=OP.mult)
                        else:
                            tmp = sb.tile([P, 512], F32, tag="tmpb")
                            nc.vector.tensor_tensor(
                                tmp[:], ps2[:],
                                fb2rep_sb[:, 512 * i:512 * (i + 1)],
                                op=OP.add)
                            hfg = _ap(gbuf[:], 4 * b * NB,
                                      [[CS * NB, P], [1, 512]])
                            nc.vector.tensor_tensor(msg[:], tmp[:], hfg,
                                                    op=OP.mult)
                        for s4 in range(4):
                            st = stb + s4
                            cki = int(pl.st_chunk[st])
                            if cki not in agg_open:
                                agg_open[cki] = ppagg.tile(
                                    [P, P], F32, tag="agg",
                                    name=f"aggps_{i}_{cki}")
                            first = (st == pl.chunk_first[cki])
                            last = (st == pl.chunk_last[cki])
                            nc.tensor.matmul(
                                agg_open[cki][:],
                                msg[:, P * s4:P * (s4 + 1)],
                                Sm[:, P * s4:P * (s4 + 1)],
                                start=first, stop=last,
                                skip_group_check=True)
                            if last:
                                nc.vector.tensor_copy(
                                    agg_sb[:, P * cki:P * (cki + 1)],
                                    agg_open[cki][:])
                                del agg_open[cki]
                                emit_update_chunk(i, cki)
                                if i + 1 < NC:
                                    emit_hf_chunk(i + 1, cki)
                                else:
                                    emit_readout_chunk(cki)

            nc.sync.dma_start(ypart[:], e_acc[:])

    # Spread gather descriptor-generation across the 4 SWDGE queues (Q7
    # core pairs), consistent with the DMASW semaphore lane Tile assigned
    # (the runtime locks each DMA semaphore to one SWDGE queue).
    import concourse.tile_sem_assignment as tsa
    sw_procs = {tsa.PROC_NAME_TO_IDX[f"DMASW{k}"]: k for k in range(8)}
    locked0 = set()
    gathers = []
    for b in nc.main_func.blocks:
        for inst in b.instructions:
            proc = getattr(inst, "bass_scheduled_proc", None)
            if proc in sw_procs:
                if isinstance(inst, mybir.InstDMAGatherAnt):
                    gathers.append((inst, sw_procs[proc]))
                else:
                    locked0.add(sw_procs[proc])
    for inst, lane in gathers:
        inst.queue_num = 0 if lane in locked0 else lane % 4

    nc.compile()
    return nc


# ----------------------------------------------------------------------------
# Entry point
# ----------------------------------------------------------------------------

_CACHE = {}


def _get_program(pl, NC, NM, coeff):
    key = (pl.n_atoms, pl.n_edges, pl.Ep, pl.K, NC, NM, round(coeff, 9))
    if key not in _CACHE:
        _CACHE[key] = build_program(pl, NC, NM, coeff)
    return _CACHE[key]


def kernel(r, xyz, a, n_per, embed, fw1, fb1, fw2, fb2, afw, afb,
           ow1, ob1, ow2, ob2, aw1, ab1, aw2, ab2, trace=False):
    r = np.asarray(r)
    xyz = np.asarray(xyz, dtype=np.float32)
    a = np.asarray(a)
    weights = dict(fw1=np.asarray(fw1), fb1=np.asarray(fb1),
                   fw2=np.asarray(fw2), fb2=np.asarray(fb2),
                   afw=np.asarray(afw), afb=np.asarray(afb),
                   ow1=np.asarray(ow1), ob1=np.asarray(ob1),
                   ow2=np.asarray(ow2), ob2=np.asarray(ob2),
                   aw1=np.asarray(aw1), ab1=np.asarray(ab1),
                   aw2=np.asarray(aw2), ab2=np.asarray(ab2))
    pl = make_plan(r, xyz, a, int(n_per), n_cores=8)
    in_maps, coeff = make_inputs(pl, r, xyz, a, np.asarray(embed), weights)
    NC = weights["fw1"].shape[0]
    nc = _get_program(pl, NC, pl.n_mol, coeff)
    res = bass_utils.run_bass_kernel_spmd(
        nc, in_maps, core_ids=list(range(pl.n_cores)), trace=trace)
    out = np.zeros(pl.n_mol, dtype=np.float64)
    for k in range(pl.n_cores):
        out += res.results[k]["ypart"][0].astype(np.float64)
    kernel._last_results = res
    return out.astype(np.float32)


# revision 14
# speedup vs baseline: 1.7736x; 1.0151x over previous
"""SchNet-style GNN message passing on 8 Trainium2 NeuronCores.

Strategy (per sharding hint): edges sharded by destination atom across 8
cores; atoms relabeled + degree-balanced so each core owns an equal shard
of destination atoms, with edges padded into a fixed, SPMD-uniform static
schedule.  Small weights replicated.  Per conv: each core computes the
atom-filter features hf = h @ afw for ITS OWN atom shard in row-major
[atoms, NB] bf16 layout, and one AllGather concatenates the shards into
the full gather table (shared scratchpad) read by hardware gather-DMA;
the scatter-add (segment sum) is done on the tensor engine as one-hot
matmuls into PSUM accumulators (edges pre-sorted by destination chunk).
Gather index streams are padded with trailing -1 entries which the SWDGE
descriptor generator skips.  Final per-molecule energies come from a mask
matmul; host sums the 8 partial [n_mol] vectors.
"""

import os
import sys
import numpy as np

sys.path.insert(0, "/opt/trn_rl_repo")

from contextlib import ExitStack

import ml_dtypes
import concourse.bass as bass
import concourse.tile as tile
import concourse.bacc as bacc
from concourse import mybir
from concourse import bass_utils

F32 = mybir.dt.float32
BF16 = mybir.dt.bfloat16
I16 = mybir.dt.int16
AF = mybir.ActivationFunctionType
OP = mybir.AluOpType

LN2 = float(np.log(2.0))
EPS = 1e-12
P = 128          # partitions / chunk size
NG = 32          # gaussians
NB = 128         # atom basis / filters
NH = 64          # readout hidden

USE_SOFTPLUS = int(os.environ.get("USE_SOFTPLUS", "0"))
NEG_IDX = int(os.environ.get("NEG_IDX", "1"))
PREFILL = int(os.environ.get("PREFILL", "1"))
SHARED_AG = int(os.environ.get("SHARED_AG", "1"))


# ----------------------------------------------------------------------------
# Host-side plan: atom relabeling, edge sharding, static schedule
# ----------------------------------------------------------------------------

class Plan:
    pass


def _greedy_pack(deg_a, deg_b, atom_ids, n_bins, rng):
    """Pack len(atom_ids) atoms into n_bins bins of exactly P atoms each,
    balancing per-bin sums of deg_a and deg_b.  Returns [n_bins, P] atom ids
    (-1 for none -> caller guarantees exact fit)."""
    n = len(atom_ids)
    assert n == n_bins * P
    tot_a = max(float(deg_a[atom_ids].sum()), 1.0)
    tot_b = max(float(deg_b[atom_ids].sum()), 1.0)
    ta = tot_a / n_bins
    tb = tot_b / n_bins
    order = np.argsort(-(deg_a[atom_ids] + deg_b[atom_ids]), kind="stable")
    sa = np.zeros(n_bins)
    sb = np.zeros(n_bins)
    cnt = np.zeros(n_bins, dtype=np.int64)
    bins = np.full((n_bins, P), -1, dtype=np.int64)
    for oi in order:
        a = atom_ids[oi]
        da, db = deg_a[a], deg_b[a]
        load = np.maximum((sa + da) / ta, (sb + db) / tb)
        load[cnt >= P] = np.inf
        i = int(np.argmin(load))
        bins[i, cnt[i]] = a
        cnt[i] += 1
        sa[i] += da
        sb[i] += db
    assert (cnt == P).all()
    return bins, sa, sb


def make_plan(r, xyz, a, n_per, n_cores=8):
    pl = Plan()
    n_atoms = xyz.shape[0]
    n_edges = a.shape[0]
    rng = np.random.default_rng(12345)

    # padded atom count: multiple of n_cores*P
    npad = ((n_atoms + n_cores * P - 1) // (n_cores * P)) * (n_cores * P)
    K = npad // (n_cores * P)          # bins (chunks) per core
    SH = K * P                          # atoms per core shard
    a_cap = min(32768, npad)            # atoms addressable by table A (idx16)
    a_cap = (a_cap // P) * P
    if a_cap < npad // 2:
        raise ValueError("atom table too large for two-base gather split")
    b_base = npad - a_cap               # B gather base row

    dst = a[:, 0].astype(np.int64)
    src = a[:, 1].astype(np.int64)

    # choose A-set (atoms whose NEW id < a_cap): random choice of a_cap reals;
    # virtual atoms (degree 0) fill whatever space remains in each group.
    n_virt = npad - n_atoms
    n_aset = min(a_cap, n_atoms)
    perm_r = rng.permutation(n_atoms)
    aset = np.zeros(n_atoms, dtype=bool)
    aset[perm_r[:n_aset]] = True

    in_a = aset[src]                    # edge half by src membership
    degA = np.bincount(dst[in_a], minlength=n_atoms)
    degB = np.bincount(dst[~in_a], minlength=n_atoms)
    degA_x = np.concatenate([degA, np.zeros(n_virt, dtype=degA.dtype)])
    degB_x = np.concatenate([degB, np.zeros(n_virt, dtype=degB.dtype)])

    virt_ids = np.arange(n_atoms, npad)
    n_virt_a = a_cap - n_aset            # virtuals needed in the A group
    a_ids = np.concatenate([np.nonzero(aset)[0], virt_ids[:n_virt_a]])
    b_ids_x = np.concatenate([np.nonzero(~aset)[0], virt_ids[n_virt_a:]])
    binsA, saA, sbA = _greedy_pack(degA_x, degB_x, a_ids, a_cap // P, rng)
    if npad > a_cap:
        binsB, saB, sbB = _greedy_pack(degA_x, degB_x, b_ids_x,
                                       (npad - a_cap) // P, rng)
    else:
        binsB = np.zeros((0, P), dtype=np.int64)
        saB = np.zeros(1)
        sbB = np.zeros(1)

    new_of_old = np.full(npad, -1, dtype=np.int64)
    allbins = np.concatenate([binsA, binsB], axis=0)    # [npad//P, P]
    flat = allbins.reshape(-1)
    new_of_old[flat] = np.arange(npad)
    old_of_new = flat                                    # new id -> old id

    maxA = int(np.maximum(saA.max(), saB.max()))
    maxB = int(np.maximum(sbA.max(), sbB.max()))
    TA = ((maxA + P - 1) // P + 3) // 4 * 4              # subtiles, mult of 4
    TA = max(TA, 4)
    TB = ((maxB + P - 1) // P + 1) // 2 * 2              # mult of 2
    TB = max(TB, 2)

    # static stream structure (identical for every core)
    groups = [(c, c + 1) for c in range(0, K - 1, 2)]
    if K % 2 == 1:
        groups.append((K - 1,))
    n_sub_main = sum(len(g) * (TA + TB) for g in groups)
    padb = (16 - (n_sub_main % 16)) % 16                # pad to 2048-edge mult
    # also need last B span % 4 == 0
    lastB = len(groups[-1]) * TB + padb
    while lastB % 4 != 0:
        padb += 16
        lastB = len(groups[-1]) * TB + padb
    n_sub = n_sub_main + padb
    Ep = n_sub * P

    # per-subtile chunk binding + gather call table (same for all cores).
    # A spans get one call per chunk, so per-chunk padding is trailing and
    # the SWDGE generator can skip it (negative idxs); B spans get one call
    # per group, split into <= CALLSUB pieces (multiples of 4 subtiles).
    CALLSUB = min(32, max(TA, 2 * TB))
    CALLSUB = (CALLSUB // 4) * 4
    st_chunk = np.zeros(n_sub, dtype=np.int64)
    calls = []            # (start_subtile, n_subtiles, half)  half: 0=A 1=B
    s = 0
    span_start = {}
    for gi, g in enumerate(groups):
        for c in g:
            span_start[(c, 0)] = s
            st_chunk[s:s + TA] = c
            off = 0
            while off < TA:
                take = min(CALLSUB, TA - off)
                calls.append((s + off, take, 0))
                off += take
            s += TA
        b0 = s
        for c in g:
            span_start[(c, 1)] = s
            st_chunk[s:s + TB] = c
            s += TB
        if gi == len(groups) - 1 and padb:
            st_chunk[s:s + padb] = g[-1]
            s += padb
        blen = s - b0
        off = 0
        while off < blen:
            take = min(CALLSUB, blen - off)
            calls.append((b0 + off, take, 1))
            off += take
    assert s == n_sub
    max_call_sub = max(ns for _, ns, _ in calls)
    assert all(ns % 4 == 0 for _, ns, _ in calls)

    # last subtile index of each chunk (for psum close / stop flag)
    chunk_last = np.zeros(K, dtype=np.int64)
    chunk_first = np.zeros(K, dtype=np.int64)
    seen = set()
    for st in range(n_sub):
        c = st_chunk[st]
        if c not in seen:
            chunk_first[c] = st
            seen.add(c)
        chunk_last[c] = st

    # ---- per-core edge data -------------------------------------------------
    src_new = new_of_old[src]
    dst_new = new_of_old[dst]
    e_core = dst_new // SH
    e_chunk = (dst_new % SH) // P
    e_half = (src_new >= a_cap).astype(np.int64)

    idx_lin = np.full((n_cores, Ep), -1 if NEG_IDX else 0, dtype=np.int16)
    dstrel_lin = np.full((n_cores, Ep), -1.0, dtype=np.float32)
    osrc_lin = np.zeros((n_cores, Ep), dtype=np.int64)
    odst_lin = np.zeros((n_cores, Ep), dtype=np.int64)

    # bucket edges by (core, chunk, half)
    order = np.lexsort((e_half, e_chunk, e_core))
    so_src, so_dst = src_new[order], dst_new[order]
    so_core, so_chunk, so_half = e_core[order], e_chunk[order], e_half[order]
    so_osrc, so_odst = src[order], dst[order]
    # boundaries
    keys = so_core * (K * 2) + so_chunk * 2 + so_half
    bstart = np.searchsorted(keys, np.arange(n_cores * K * 2), side="left")
    bend = np.searchsorted(keys, np.arange(n_cores * K * 2), side="right")

    for core in range(n_cores):
        for c in range(K):
            for h in (0, 1):
                bi = core * (K * 2) + c * 2 + h
                e0, e1 = bstart[bi], bend[bi]
                cnt = e1 - e0
                cap = (TA if h == 0 else TB) * P
                assert cnt <= cap, (core, c, h, cnt, cap)
                p0 = span_start[(c, h)] * P
                sl = slice(p0, p0 + cnt)
                if h == 0:
                    idx_lin[core, sl] = so_src[e0:e1].astype(np.int16)
                else:
                    idx_lin[core, sl] = (so_src[e0:e1] - b_base).astype(np.int16)
                dstrel_lin[core, sl] = (so_dst[e0:e1] % P).astype(np.float32)
                osrc_lin[core, sl] = so_osrc[e0:e1]
                odst_lin[core, sl] = so_odst[e0:e1]

    # interior padding (pad rows followed by a real row within the same
    # call) must gather a safe row (0); only trailing pads stay -1 so the
    # SWDGE descriptor generator drops them.
    # Per-call gather count, uniform across cores (SPMD): the value-trim in
    # the SWDGE ucode and the ring-space reservation in the decode stage must
    # agree, and the decode uses num_idxs_reg.  cnt = max real rows over
    # cores (16-rounded); pads below cnt gather row 0, rows >= cnt are -1 and
    # are skipped by every core identically.
    call_cnt = []
    for (st0, nsx, half) in calls:
        r0, r1 = st0 * P, (st0 + nsx) * P
        cnt = 0
        for core in range(n_cores):
            real = np.nonzero(dstrel_lin[core, r0:r1] >= 0)[0]
            if len(real):
                cnt = max(cnt, int(real[-1]) + 1)
        cnt = min((cnt + 15) // 16 * 16, nsx * P)
        call_cnt.append(cnt)
        if NEG_IDX:
            for core in range(n_cores):
                seg = idx_lin[core, r0:r0 + cnt]
                seg[dstrel_lin[core, r0:r0 + cnt] < 0] = 0
            idx_lin[:, r0 + cnt:r1] = -1
        else:
            call_cnt[-1] = nsx * P
            for core in range(n_cores):
                seg = idx_lin[core, r0:r1]
                seg[dstrel_lin[core, r0:r1] < 0] = 0

    pl.n_atoms, pl.n_edges, pl.npad = n_atoms, n_edges, npad
    pl.n_cores, pl.K, pl.SH, pl.Ep, pl.n_sub = n_cores, K, SH, Ep, n_sub
    pl.TA, pl.TB, pl.padb = TA, TB, padb
    pl.a_cap, pl.b_base = a_cap, b_base
    pl.groups, pl.calls, pl.max_call_sub = groups, calls, max_call_sub
    pl.call_cnt = call_cnt
    pl.st_chunk, pl.chunk_first, pl.chunk_last = st_chunk, chunk_first, chunk_last
    pl.new_of_old, pl.old_of_new = new_of_old, old_of_new
    pl.idx_lin, pl.dstrel_lin = idx_lin, dstrel_lin
    pl.osrc_lin, pl.odst_lin = osrc_lin, odst_lin
    pl.n_per = int(n_per)
    pl.n_mol = n_atoms // pl.n_per
    return pl


def make_inputs(pl, r, xyz, a, embed, weights):
    """Build per-core in_maps.  weights: dict of raw weight arrays."""
    C, K, SH, Ep, n_sub = pl.n_cores, pl.K, pl.SH, pl.Ep, pl.n_sub
    NC = weights["fw1"].shape[0]
    NM = pl.n_mol
    F0 = Ep // P

    h0_all = embed[r[:, 0].astype(np.int64)].astype(np.float32)     # [n,NB]
    h0_new = np.zeros((pl.npad, NB), dtype=np.float32)
    real = pl.old_of_new < pl.n_atoms
    h0_new[real] = h0_all[pl.old_of_new[real]]

    mol_new = np.full(pl.npad, -1, dtype=np.int64)
    mol_new[real] = pl.old_of_new[real] // pl.n_per

    xyzf = xyz.astype(np.float32)

    fw1, fb1 = weights["fw1"], weights["fb1"]
    fw2, fb2 = weights["fw2"], weights["fb2"]
    afw, afb = weights["afw"], weights["afb"]
    ow1, ob1 = weights["ow1"], weights["ob1"]
    ow2, ob2 = weights["ow2"], weights["ob2"]
    aw1, ab1 = weights["aw1"], weights["ab1"]
    aw2, ab2 = weights["aw2"], weights["ab2"]
    assert np.all(afb == 0.0), "nonzero afb not supported by this kernel"

    # fold ssp's -log(2) into the following layer's bias
    fb2e = (fb2 - LN2 * fw2.sum(axis=1)).astype(np.float32)         # [NC,NB]
    ob2e = (ob2 - LN2 * ow2.sum(axis=1)).astype(np.float32)         # [NC,NB]
    ab2e = float(ab2[0] - LN2 * aw2.sum(axis=0)[0])

    offs = np.linspace(0.0, 5.0, NG).astype(np.float32)
    width = float(offs[1] - offs[0])
    coeff = -0.5 / (width * width)

    # fb2 replicated across partitions: [P, NC*4*NB]
    fb2rep = np.concatenate([np.tile(fb2e[i], (P, 4)) for i in range(NC)],
                            axis=1).astype(np.float32)

    shared = {
        "fw1b": np.ascontiguousarray(
            fw1.transpose(1, 0, 2).reshape(NG, NC * NB)).astype(ml_dtypes.bfloat16),
        "fw2b": np.ascontiguousarray(
            fw2.transpose(1, 0, 2).reshape(NB, NC * NB)).astype(ml_dtypes.bfloat16),
        "afwb": np.ascontiguousarray(
            afw.transpose(1, 0, 2).reshape(NB, NC * NB)).astype(ml_dtypes.bfloat16),
        "ow1w": np.ascontiguousarray(
            ow1.transpose(1, 0, 2).reshape(NB, NC * NB)).astype(np.float32),
        "ow2w": np.ascontiguousarray(
            ow2.transpose(1, 0, 2).reshape(NB, NC * NB)).astype(np.float32),
        "aw1w": aw1.astype(np.float32),                              # [NB,NH]
        "aw2w": aw2.astype(np.float32),                              # [NH,1]
        "fb1t": np.ascontiguousarray(fb1.T).astype(np.float32),      # [NB,NC]
        "ob1t": np.ascontiguousarray(ob1.T).astype(np.float32),
        "ob2et": np.ascontiguousarray(ob2e.T).astype(np.float32),
        "fb2rep": fb2rep,                                            # [P,NC*512]
        "ab1t": ab1.reshape(NH, 1).astype(np.float32),
        "ab2p": np.full((P, 1), ab2e, dtype=np.float32),
        "negmu": np.tile(-offs, 4).reshape(P, 1).astype(np.float32),
        "iota512": np.tile(np.arange(P, dtype=np.float32), (P, 4)),
    }

    in_maps = []
    for c in range(C):
        m = dict(shared)
        osrc = pl.osrc_lin[c]
        odst = pl.odst_lin[c]
        xs = xyzf[osrc]          # [Ep,3]
        xd = xyzf[odst]
        m["xsx"] = np.ascontiguousarray(xs[:, 0].reshape(P, F0))
        m["xsy"] = np.ascontiguousarray(xs[:, 1].reshape(P, F0))
        m["xsz"] = np.ascontiguousarray(xs[:, 2].reshape(P, F0))
        m["xdx"] = np.ascontiguousarray(xd[:, 0].reshape(P, F0))
        m["xdy"] = np.ascontiguousarray(xd[:, 1].reshape(P, F0))
        m["xdz"] = np.ascontiguousarray(xd[:, 2].reshape(P, F0))
        m["idx"] = np.ascontiguousarray(
            np.tile(pl.idx_lin[c].reshape(Ep // 16, 16).T, (8, 1)))
        m["dstrel"] = np.ascontiguousarray(
            pl.dstrel_lin[c].reshape(n_sub, P).T)
        m["h0t"] = np.ascontiguousarray(
            h0_new[c * SH:(c + 1) * SH].T)                          # [NB,SH]
        msk = np.zeros((K, P, NM), dtype=np.float32)
        mols = mol_new[c * SH:(c + 1) * SH].reshape(K, P)
        for mm in range(NM):
            msk[:, :, mm] = (mols == mm)
        m["mask"] = msk
        in_maps.append(m)
    return in_maps, coeff


# ----------------------------------------------------------------------------
# Device program
# ----------------------------------------------------------------------------

def _ap(tile_ap, extra_off, pattern):
    """Raw access-pattern surgery on a (pool-tile or dram) AP."""
    return bass.AP(tile_ap.tensor, tile_ap.offset + extra_off, pattern)


def _patch_act_tables():
    """Pin each activation function to exactly one ACT table so bacc never
    thrashes table loads: Softplus/Copy/Identity -> softplus_and_others
    (Softplus is missing from act_info's listing but present in the HW
    table), Exp -> exp_and_others, Sqrt -> sqrt_and_others."""
    if getattr(bacc, "_act_tables_patched", False):
        return
    orig = bacc.get_activation_tables

    if USE_SOFTPLUS:
        def patched(arch):
            t = dict(orig(arch))
            shared = {AF.Identity, AF.Copy, AF.Square}
            for name in list(t):
                s = set(t[name])
                if name == "softplus_and_others":
                    s |= {AF.Softplus}
                else:
                    s -= shared | {AF.Softplus}
                if name != "exp_and_others":
                    s -= {AF.Exp}
                if name != "sqrt_and_others":
                    s -= {AF.Sqrt}
                t[name] = s
            return t
    else:
        def patched(arch):
            t = dict(orig(arch))
            shared = {AF.Exp, AF.Ln, AF.Identity, AF.Copy, AF.Square}
            for name in list(t):
                if name != "natural_log_exp_and_others":
                    t[name] = t[name] - shared
            return t

    bacc.get_activation_tables = patched
    bacc._act_tables_patched = True


def build_program(pl, NC, NM, coeff):
    _patch_act_tables()
    C, K, SH, Ep, n_sub = pl.n_cores, pl.K, pl.SH, pl.Ep, pl.n_sub
    F0 = Ep // P
    Q = Ep // 4                      # edges per gaussian partition-group
    NW = 4                           # phase-0 g-build col iterations
    while Q % NW != 0 or (Q // NW) > 1024:
        NW *= 2
    Wg = Q // NW
    CS = pl.max_call_sub

    nc = bacc.Bacc("TRN2", target_bir_lowering=False, debug=False,
                   enable_asserts=False, num_devices=C, num_swdge_queues=4,
                   dynamic_dma_scratch_size=int(os.environ.get("DMA_SCRATCH", "16384")))

    def din(name, shape, dt=F32):
        return nc.dram_tensor(name, shape, dt, kind="ExternalInput").ap()

    xsx, xsy, xsz = din("xsx", [P, F0]), din("xsy", [P, F0]), din("xsz", [P, F0])
    xdx, xdy, xdz = din("xdx", [P, F0]), din("xdy", [P, F0]), din("xdz", [P, F0])
    idx_d = din("idx", [P, Ep // 16], I16)
    dstrel_d = din("dstrel", [P, n_sub])
    h0t_d = din("h0t", [NB, SH])
    mask_d = din("mask", [K, P, NM])
    fw1b_d = din("fw1b", [NG, NC * NB], BF16)
    fw2b_d = din("fw2b", [NB, NC * NB], BF16)
    afwb_d = din("afwb", [NB, NC * NB], BF16)
    ow1w_d = din("ow1w", [NB, NC * NB])
    ow2w_d = din("ow2w", [NB, NC * NB])
    aw1w_d = din("aw1w", [NB, NH])
    aw2w_d = din("aw2w", [NH, 1])
    fb1t_d = din("fb1t", [NB, NC])
    ob1t_d = din("ob1t", [NB, NC])
    ob2et_d = din("ob2et", [NB, NC])
    fb2rep_d = din("fb2rep", [P, NC * 4 * NB])
    ab1t_d = din("ab1t", [NH, 1])
    ab2p_d = din("ab2p", [P, 1])
    negmu_d = din("negmu", [P, 1])
    iota512_d = din("iota512", [P, 4 * P])

    ypart = nc.dram_tensor("ypart", [1, NM], F32, kind="ExternalOutput").ap()

    with tile.TileContext(nc) as tc:
        with ExitStack() as ctx:
            dram = ctx.enter_context(tc.tile_pool(name="dram", bufs=1, space="DRAM"))
            res = ctx.enter_context(tc.tile_pool(name="res", bufs=1))
            sb = ctx.enter_context(tc.tile_pool(name="sb", bufs=3))
            gpool = ctx.enter_context(tc.tile_pool(name="gpool", bufs=5))
            p0 = ctx.enter_context(tc.tile_pool(name="p0", bufs=2))
            spool = ctx.enter_context(tc.tile_pool(name="spool", bufs=3))
            pps = ctx.enter_context(tc.tile_pool(name="pps", bufs=2, space="PSUM"))
            ppagg = ctx.enter_context(tc.tile_pool(name="ppagg", bufs=2, space="PSUM"))
            ppu = ctx.enter_context(tc.tile_pool(name="ppu", bufs=2, space="PSUM"))

            # ---- DRAM scratch ----
            d_dram = dram.tile([P, F0], F32)
            g_dram = dram.tile([4, NG, Q], BF16)
            ag_space = "Shared" if SHARED_AG else "Local"
            hf_my = [dram.tile([SH, NB], BF16, name=f"hf_my{i}")
                     for i in range(NC)]
            hf_tab = [dram.tile([pl.npad, NB], BF16, addr_space=ag_space,
                                name=f"hf_tab{i}")
                      for i in range(NC)]

            # ---- resident SBUF ----
            h_my = res.tile([NB, SH], F32)
            agg_sb = res.tile([NB, SH], F32)
            idx_sb = res.tile([P, Ep // 16], I16)
            dstrel_sb = res.tile([P, n_sub], F32)
            iota_sb = res.tile([P, 4 * P], F32)
            mask_sb = res.tile([P, K * NM], F32)
            fw1b_sb = res.tile([NG, NC * NB], BF16)
            fw2b_sb = res.tile([NB, NC * NB], BF16)
            afwb_sb = res.tile([NB, NC * NB], BF16)
            ow1_sb = res.tile([NB, NC * NB], F32)
            ow2_sb = res.tile([NB, NC * NB], F32)
            aw1_sb = res.tile([NB, NH], F32)
            aw2_sb = res.tile([NH, 1], F32)
            fb1_sb = res.tile([NB, NC], F32)
            ob1_sb = res.tile([NB, NC], F32)
            ob2e_sb = res.tile([NB, NC], F32)
            fb2rep_sb = res.tile([P, NC * 4 * NB], F32)
            ab1_sb = res.tile([NH, 1], F32)
            ab2p_sb = res.tile([P, 1], F32)
            negmu_sb = res.tile([P, 1], F32)
            epsb_sb = res.tile([P, 1], F32)
            e_acc = res.tile([1, NM], F32)
            nc.vector.memset(epsb_sb[:], EPS)
            nc.vector.memset(e_acc[:], 0.0)

            nc.sync.dma_start(h_my[:], h0t_d[:])
            nc.sync.dma_start(idx_sb[:], idx_d[:])
            nc.sync.dma_start(dstrel_sb[:], dstrel_d[:])
            nc.sync.dma_start(iota_sb[:], iota512_d[:])
            # mask [K,P,NM] -> [P, K*NM]
            nc.sync.dma_start(
                _ap(mask_sb[:], 0, [[K * NM, P], [NM, K], [1, NM]]),
                _ap(mask_d, 0, [[NM, P], [P * NM, K], [1, NM]]))
            for t_sb, t_d in [(fw1b_sb, fw1b_d), (fw2b_sb, fw2b_d),
                              (afwb_sb, afwb_d), (ow1_sb, ow1w_d),
                              (ow2_sb, ow2w_d), (aw1_sb, aw1w_d),
                              (aw2_sb, aw2w_d), (fb1_sb, fb1t_d),
                              (ob1_sb, ob1t_d), (ob2e_sb, ob2et_d),
                              (fb2rep_sb, fb2rep_d), (ab1_sb, ab1t_d),
                              (ab2p_sb, ab2p_d), (negmu_sb, negmu_d)]:
                nc.sync.dma_start(t_sb[:], t_d[:])

            # zero-init gather buffers once so skipped (trailing-pad) rows
            # always hold finite values
            for z in range(5):
                gz = gpool.tile([P, CS * NB], BF16, tag="gbuf", name=f"gz{z}")
                nc.vector.memset(gz[:], 0.0)

            def emit_hf_chunk(i, c):
                """hf rows for chunk c of conv i from current h_my."""
                hb = sb.tile([NB, P], BF16, tag="hb")
                if int(os.environ.get("HB_SCALAR", "1")):
                    nc.scalar.copy(hb[:], h_my[:, P * c:P * (c + 1)])
                else:
                    nc.vector.tensor_copy(hb[:], h_my[:, P * c:P * (c + 1)])
                hfps = ppu.tile([P, P], F32, tag="upd", name=f"hfps_{i}_{c}")
                nc.tensor.matmul(hfps[:], hb[:],
                                 afwb_sb[:, NB * i:NB * (i + 1)],
                                 start=True, stop=True)
                hfsb = sb.tile([P, P], BF16, tag="hfsb")
                nc.scalar.copy(hfsb[:], hfps[:])
                nc.sync.dma_start(hf_my[i][P * c:P * (c + 1), :], hfsb[:])

            def emit_update_chunk(i, c):
                """h += dense(ssp(dense(agg)))  for chunk c, conv i."""
                ups = ppu.tile([P, P], F32, tag="upd", name=f"ups_{i}_{c}")
                nc.tensor.matmul(ups[:], ow1_sb[:, NB * i:NB * (i + 1)],
                                 agg_sb[:, P * c:P * (c + 1)],
                                 start=True, stop=True)
                usb = sb.tile([P, P], F32, tag="usb")
                if USE_SOFTPLUS:
                    nc.scalar.activation(usb[:], ups[:], AF.Softplus,
                                         bias=ob1_sb[:, i:i + 1], scale=1.0)
                else:
                    ue = sb.tile([P, P], F32, tag="ue")
                    nc.scalar.activation(ue[:], ups[:], AF.Exp,
                                         bias=ob1_sb[:, i:i + 1], scale=1.0)
                    nc.scalar.activation(usb[:], ue[:], AF.Ln,
                                         bias=1.0, scale=1.0)
                drps = ppu.tile([P, P], F32, tag="upd", name=f"drps_{i}_{c}")
                nc.tensor.matmul(drps[:], ow2_sb[:, NB * i:NB * (i + 1)],
                                 usb[:], start=True, stop=True)
                drt = sb.tile([P, P], F32, tag="drt")
                nc.vector.tensor_scalar(drt[:], drps[:],
                                        ob2e_sb[:, i:i + 1], None,
                                        op0=OP.add)
                nc.vector.tensor_add(h_my[:, P * c:P * (c + 1)],
                                     h_my[:, P * c:P * (c + 1)], drt[:])

            def emit_readout_chunk(c):
                r1ps = ppu.tile([NH, P], F32, tag="upd", name=f"r1ps{c}")
                nc.tensor.matmul(r1ps[:], aw1_sb[:],
                                 h_my[:, P * c:P * (c + 1)],
                                 start=True, stop=True)
                r1sb = sb.tile([NH, P], F32, tag="r1sb")
                if USE_SOFTPLUS:
                    nc.scalar.activation(r1sb[:], r1ps[:], AF.Softplus,
                                         bias=ab1_sb[:, 0:1], scale=1.0)
                else:
                    r1e = sb.tile([NH, P], F32, tag="r1e")
                    nc.scalar.activation(r1e[:], r1ps[:], AF.Exp,
                                         bias=ab1_sb[:, 0:1], scale=1.0)
                    nc.scalar.activation(r1sb[:], r1e[:], AF.Ln,
                                         bias=1.0, scale=1.0)
                yps = ppu.tile([P, 1], F32, tag="upd", name=f"yps{c}")
                nc.tensor.matmul(yps[:], r1sb[:], aw2_sb[:],
                                 start=True, stop=True)
                ysb = sb.tile([P, 1], F32, tag="ysb")
                nc.scalar.activation(ysb[:], yps[:], AF.Identity,
                                     bias=ab2p_sb[:, 0:1], scale=1.0)
                em_ps = ppu.tile([1, NM], F32, tag="upd", name=f"emps{c}")
                nc.tensor.matmul(em_ps[:], ysb[:],
                                 mask_sb[:, NM * c:NM * (c + 1)],
                                 start=True, stop=True)
                nc.vector.tensor_add(e_acc[:], e_acc[:], em_ps[:])

            # ================= phase 0: distances and gaussians ============
            cx = p0.tile([P, F0], F32, tag="ph0", bufs=1)
            cy = p0.tile([P, F0], F32, tag="ph0b", bufs=1)
            cz = p0.tile([P, F0], F32, tag="ph0c", bufs=1)
            tx = p0.tile([P, F0], F32, tag="ph0d", bufs=1)
            nc.sync.dma_start(cx[:], xsx[:])
            nc.sync.dma_start(tx[:], xdx[:])
            nc.vector.tensor_sub(cx[:], cx[:], tx[:])
            nc.vector.tensor_mul(cx[:], cx[:], cx[:])
            nc.sync.dma_start(cy[:], xsy[:])
            nc.sync.dma_start(tx[:], xdy[:])
            nc.vector.tensor_sub(cy[:], cy[:], tx[:])
            nc.vector.tensor_mul(cy[:], cy[:], cy[:])
            nc.sync.dma_start(cz[:], xsz[:])
            nc.sync.dma_start(tx[:], xdz[:])
            nc.vector.tensor_sub(cz[:], cz[:], tx[:])
            nc.vector.tensor_mul(cz[:], cz[:], cz[:])
            nc.vector.tensor_add(cx[:], cx[:], cy[:])
            nc.vector.tensor_add(cx[:], cx[:], cz[:])
            nc.scalar.activation(cy[:], cx[:], AF.Sqrt,
                                 bias=epsb_sb[:, 0:1], scale=1.0)
            nc.sync.dma_start(d_dram[:], cy[:])

            # startup: hf table for conv 0 from h0 (emitted after the coord
            # chain so phase-0 DVE work isn't queued behind the hf copies)
            for c in range(K):
                emit_hf_chunk(0, c)

            for w in range(NW):
                dbc = p0.tile([P, Wg], F32, tag="dbc")
                nc.sync.dma_start(
                    dbc[:], _ap(d_dram[:], w * Wg, [[Q, 4], [0, NG], [1, Wg]]))
                t1 = p0.tile([P, Wg], F32, tag="t1")
                nc.vector.tensor_scalar(t1[:], dbc[:], negmu_sb[:, 0:1], None,
                                        op0=OP.add)
                nc.vector.tensor_mul(t1[:], t1[:], t1[:])
                gt = p0.tile([P, Wg], BF16, tag="gt0")
                nc.scalar.activation(gt[:], t1[:], AF.Exp, bias=0.0, scale=coeff)
                nc.sync.dma_start(
                    _ap(g_dram[:], w * Wg, [[NG * Q, 4], [Q, NG], [1, Wg]]),
                    gt[:])

            # ================= conv layers =================================
            for i in range(NC):
                # --- allgather hf shards into the full table ------------
                nc.gpsimd.collective_compute(
                    "AllGather", OP.bypass,
                    replica_groups=[list(range(C))],
                    ins=[hf_my[i].opt()], outs=[hf_tab[i].opt()])

                # --- edge phase ---------------------------------------
                agg_open = {}
                for ci, (st0, nsx, half) in enumerate(pl.calls):
                    gbuf = gpool.tile([P, CS * NB], BF16, tag="gbuf")
                    if half == 0:
                        tbl_ap = _ap(hf_tab[i][:], 0,
                                     [[NB, pl.a_cap], [1, NB]])
                    else:
                        tbl_ap = _ap(hf_tab[i][:], pl.b_base * NB,
                                     [[NB, pl.npad - pl.b_base], [1, NB]])
                    nc.gpsimd.dma_gather(
                        _ap(gbuf[:], 0, [[CS * NB, P], [NB, nsx], [1, NB]]),
                        tbl_ap,
                        idx_sb[:, 8 * st0:8 * (st0 + nsx)],
                        P * nsx, pl.call_cnt[ci], NB, single_packet=False)

                    for b in range(nsx // 4):
                        stb = st0 + 4 * b
                        e0 = stb * P
                        q, col = e0 // Q, e0 % Q
                        gt2 = sb.tile([NG, 512], BF16, tag="gt2")
                        nc.sync.dma_start(gt2[:], g_dram[q, :, col:col + 512])
                        ps1 = pps.tile([P, 512], F32, tag="ps1")
                        nc.tensor.matmul(ps1[:],
                                         fw1b_sb[:, NB * i:NB * (i + 1)],
                                         gt2[:], start=True, stop=True)
                        x1 = sb.tile([P, 512], BF16, tag="x1")
                        if USE_SOFTPLUS:
                            nc.scalar.activation(x1[:], ps1[:], AF.Softplus,
                                                 bias=fb1_sb[:, i:i + 1],
                                                 scale=1.0)
                        else:
                            nc.scalar.activation(ps1[:], ps1[:], AF.Exp,
                                                 bias=fb1_sb[:, i:i + 1],
                                                 scale=1.0)
                            nc.scalar.activation(x1[:], ps1[:], AF.Ln,
                                                 bias=1.0, scale=1.0)
                        ps2 = pps.tile([P, 512], F32, tag="ps2")
                        if PREFILL:
                            nc.scalar.copy(
                                ps2[:], fb2rep_sb[:, 512 * i:512 * (i + 1)])
                        for s4 in range(4):
                            nc.tensor.matmul(
                                ps2[:, P * s4:P * (s4 + 1)],
                                x1[:, P * s4:P * (s4 + 1)],
                                fw2b_sb[:, NB * i:NB * (i + 1)],
                                start=not PREFILL, stop=True,
                                skip_group_check=True)
                        Sm = spool.tile([P, 512], BF16, tag="Sm")
                        dr_ap = _ap(dstrel_sb[:], stb,
                                    [[n_sub, P], [1, 4], [0, P]])
                        nc.vector.tensor_tensor(Sm[:], iota_sb[:], dr_ap,
                                                op=OP.is_equal)
                        msg = sb.tile([P, 512], BF16, tag="msg")
                        if PREFILL:
                            hfg = _ap(gbuf[:], 4 * b * NB,
                                      [[CS * NB, P], [1, 512]])
                            nc.vector.tensor_tensor(msg[:], ps2[:], hfg,
                                                    op=OP.mult)
                        else:
                            tmp = sb.tile([P, 512], F32, tag="tmpb")
                            nc.vector.tensor_tensor(
                                tmp[:], ps2[:],
                                fb2rep_sb[:, 512 * i:512 * (i + 1)],
                                op=OP.add)
                            hfg = _ap(gbuf[:], 4 * b * NB,
                                      [[CS * NB, P], [1, 512]])
                            nc.vector.tensor_tensor(msg[:], tmp[:], hfg,
                                                    op=OP.mult)
                        for s4 in range(4):
                            st = stb + s4
                            cki = int(pl.st_chunk[st])
                            if cki not in agg_open:
                                agg_open[cki] = ppagg.tile(
                                    [P, P], F32, tag="agg",
                                    name=f"aggps_{i}_{cki}")
                            first = (st == pl.chunk_first[cki])
                            last = (st == pl.chunk_last[cki])
                            nc.tensor.matmul(
                                agg_open[cki][:],
                                msg[:, P * s4:P * (s4 + 1)],
                                Sm[:, P * s4:P * (s4 + 1)],
                                start=first, stop=last,
                                skip_group_check=True)
                            if last:
                                nc.vector.tensor_copy(
                                    agg_sb[:, P * cki:P * (cki + 1)],
                                    agg_open[cki][:])
                                del agg_open[cki]
                                emit_update_chunk(i, cki)
                                if i + 1 < NC:
                                    emit_hf_chunk(i + 1, cki)
                                else:
                                    emit_readout_chunk(cki)

            nc.sync.dma_start(ypart[:], e_acc[:])

    # Spread gather descriptor-generation across the 4 SWDGE queues (Q7
    # core pairs), consistent with the DMASW semaphore lane Tile assigned
    # (the runtime locks each DMA semaphore to one SWDGE queue).
    import concourse.tile_sem_assignment as tsa
    sw_procs = {tsa.PROC_NAME_TO_IDX[f"DMASW{k}"]: k for k in range(8)}
    locked0 = set()
    gathers = []
    for b in nc.main_func.blocks:
        for inst in b.instructions:
            proc = getattr(inst, "bass_scheduled_proc", None)
            if proc in sw_procs:
                if isinstance(inst, mybir.InstDMAGatherAnt):
                    gathers.append((inst, sw_procs[proc]))
                else:
                    locked0.add(sw_procs[proc])
    for inst, lane in gathers:
        inst.queue_num = 0 if lane in locked0 else lane % 4

    nc.compile()
    return nc


# ----------------------------------------------------------------------------
# Entry point
# ----------------------------------------------------------------------------

_CACHE = {}


def _get_program(pl, NC, NM, coeff):
    key = (pl.n_atoms, pl.n_edges, pl.Ep, pl.K, NC, NM, round(coeff, 9))
    if key not in _CACHE:
        _CACHE[key] = build_program(pl, NC, NM, coeff)
    return _CACHE[key]


def kernel(r, xyz, a, n_per, embed, fw1, fb1, fw2, fb2, afw, afb,
           ow1, ob1, ow2, ob2, aw1, ab1, aw2, ab2, trace=False):
    r = np.asarray(r)
    xyz = np.asarray(xyz, dtype=np.float32)
    a = np.asarray(a)
    weights = dict(fw1=np.asarray(fw1), fb1=np.asarray(fb1),
                   fw2=np.asarray(fw2), fb2=np.asarray(fb2),
                   afw=np.asarray(afw), afb=np.asarray(afb),
                   ow1=np.asarray(ow1), ob1=np.asarray(ob1),
                   ow2=np.asarray(ow2), ob2=np.asarray(ob2),
                   aw1=np.asarray(aw1), ab1=np.asarray(ab1),
                   aw2=np.asarray(aw2), ab2=np.asarray(ab2))
    pl = make_plan(r, xyz, a, int(n_per), n_cores=8)
    in_maps, coeff = make_inputs(pl, r, xyz, a, np.asarray(embed), weights)
    NC = weights["fw1"].shape[0]
    nc = _get_program(pl, NC, pl.n_mol, coeff)
    res = bass_utils.run_bass_kernel_spmd(
        nc, in_maps, core_ids=list(range(pl.n_cores)), trace=trace)
    out = np.zeros(pl.n_mol, dtype=np.float64)
    for k in range(pl.n_cores):
        out += res.results[k]["ypart"][0].astype(np.float64)
    kernel._last_results = res
    return out.astype(np.float32)


# revision 17
# speedup vs baseline: 2.0763x; 1.1707x over previous
"""SchNet-style GNN message passing on 8 Trainium2 NeuronCores.

Strategy (per sharding hint): edges sharded by destination atom across 8
cores; atoms relabeled + degree-balanced so each core owns an equal shard
of destination atoms, with edges padded into a fixed, SPMD-uniform static
schedule.  Small weights replicated.  Per conv: each core computes the
atom-filter features hf = h @ afw for ITS OWN atom shard in row-major
[atoms, NB] bf16 layout, and one AllGather concatenates the shards into
the full gather table (shared scratchpad) read by hardware gather-DMA;
the scatter-add (segment sum) is done on the tensor engine as one-hot
matmuls into PSUM accumulators (edges pre-sorted by destination chunk).
Gather index streams are padded with trailing -1 entries which the SWDGE
descriptor generator skips.  Final per-molecule energies come from a mask
matmul; host sums the 8 partial [n_mol] vectors.
"""

import os
import sys
import numpy as np

os.environ.setdefault("NEURON_RT_RESET_CORES", "1")
sys.path.insert(0, "/opt/trn_rl_repo")

from contextlib import ExitStack

import ml_dtypes
import concourse.bass as bass
import concourse.tile as tile
import concourse.bacc as bacc
from concourse import mybir
from concourse import bass_utils

F32 = mybir.dt.float32
BF16 = mybir.dt.bfloat16
I16 = mybir.dt.int16
AF = mybir.ActivationFunctionType
OP = mybir.AluOpType

LN2 = float(np.log(2.0))
EPS = 1e-12
P = 128          # partitions / chunk size
NG = 32          # gaussians
NB = 128         # atom basis / filters
NH = 64          # readout hidden

USE_SOFTPLUS = int(os.environ.get("USE_SOFTPLUS", "0"))
NEG_IDX = int(os.environ.get("NEG_IDX", "1"))
PREP_N = int(os.environ.get("PREP_N", "0"))
PREFILL = int(os.environ.get("PREFILL", "1"))
SHARED_AG = int(os.environ.get("SHARED_AG", "1"))


# ----------------------------------------------------------------------------
# Host-side plan: atom relabeling, edge sharding, static schedule
# ----------------------------------------------------------------------------

class Plan:
    pass


def _greedy_pack(deg_a, deg_b, atom_ids, n_bins, rng):
    """Pack len(atom_ids) atoms into n_bins bins of exactly P atoms each,
    balancing per-bin sums of deg_a and deg_b.  Returns [n_bins, P] atom ids
    (-1 for none -> caller guarantees exact fit)."""
    n = len(atom_ids)
    assert n == n_bins * P
    tot_a = max(float(deg_a[atom_ids].sum()), 1.0)
    tot_b = max(float(deg_b[atom_ids].sum()), 1.0)
    ta = tot_a / n_bins
    tb = tot_b / n_bins
    order = np.argsort(-(deg_a[atom_ids] + deg_b[atom_ids]), kind="stable")
    sa = np.zeros(n_bins)
    sb = np.zeros(n_bins)
    cnt = np.zeros(n_bins, dtype=np.int64)
    bins = np.full((n_bins, P), -1, dtype=np.int64)
    for oi in order:
        a = atom_ids[oi]
        da, db = deg_a[a], deg_b[a]
        load = np.maximum((sa + da) / ta, (sb + db) / tb)
        load[cnt >= P] = np.inf
        i = int(np.argmin(load))
        bins[i, cnt[i]] = a
        cnt[i] += 1
        sa[i] += da
        sb[i] += db
    assert (cnt == P).all()
    return bins, sa, sb


def make_plan(r, xyz, a, n_per, n_cores=8):
    pl = Plan()
    n_atoms = xyz.shape[0]
    n_edges = a.shape[0]
    rng = np.random.default_rng(12345)

    # padded atom count: multiple of n_cores*P
    npad = ((n_atoms + n_cores * P - 1) // (n_cores * P)) * (n_cores * P)
    K = npad // (n_cores * P)          # bins (chunks) per core
    SH = K * P                          # atoms per core shard
    a_cap = min(32768, npad)            # atoms addressable by table A (idx16)
    a_cap = (a_cap // P) * P
    if a_cap < npad // 2:
        raise ValueError("atom table too large for two-base gather split")
    b_base = npad - a_cap               # B gather base row

    dst = a[:, 0].astype(np.int64)
    src = a[:, 1].astype(np.int64)

    # choose A-set (atoms whose NEW id < a_cap): random choice of a_cap reals;
    # virtual atoms (degree 0) fill whatever space remains in each group.
    n_virt = npad - n_atoms
    n_aset = min(a_cap, n_atoms)
    perm_r = rng.permutation(n_atoms)
    aset = np.zeros(n_atoms, dtype=bool)
    aset[perm_r[:n_aset]] = True

    in_a = aset[src]                    # edge half by src membership
    degA = np.bincount(dst[in_a], minlength=n_atoms)
    degB = np.bincount(dst[~in_a], minlength=n_atoms)
    degA_x = np.concatenate([degA, np.zeros(n_virt, dtype=degA.dtype)])
    degB_x = np.concatenate([degB, np.zeros(n_virt, dtype=degB.dtype)])

    virt_ids = np.arange(n_atoms, npad)
    n_virt_a = a_cap - n_aset            # virtuals needed in the A group
    a_ids = np.concatenate([np.nonzero(aset)[0], virt_ids[:n_virt_a]])
    b_ids_x = np.concatenate([np.nonzero(~aset)[0], virt_ids[n_virt_a:]])
    binsA, saA, sbA = _greedy_pack(degA_x, degB_x, a_ids, a_cap // P, rng)
    if npad > a_cap:
        binsB, saB, sbB = _greedy_pack(degA_x, degB_x, b_ids_x,
                                       (npad - a_cap) // P, rng)
    else:
        binsB = np.zeros((0, P), dtype=np.int64)
        saB = np.zeros(1)
        sbB = np.zeros(1)

    new_of_old = np.full(npad, -1, dtype=np.int64)
    allbins = np.concatenate([binsA, binsB], axis=0)    # [npad//P, P]
    flat = allbins.reshape(-1)
    new_of_old[flat] = np.arange(npad)
    old_of_new = flat                                    # new id -> old id

    maxA = int(np.maximum(saA.max(), saB.max()))
    maxB = int(np.maximum(sbA.max(), sbB.max()))
    TA = ((maxA + P - 1) // P + 3) // 4 * 4              # subtiles, mult of 4
    TA = max(TA, 4)
    TB = ((maxB + P - 1) // P + 1) // 2 * 2              # mult of 2
    TB = max(TB, 2)

    # static stream structure (identical for every core)
    groups = [(c, c + 1) for c in range(0, K - 1, 2)]
    if K % 2 == 1:
        groups.append((K - 1,))
    n_sub_main = sum(len(g) * (TA + TB) for g in groups)
    padb = (16 - (n_sub_main % 16)) % 16                # pad to 2048-edge mult
    # also need last B span % 4 == 0
    lastB = len(groups[-1]) * TB + padb
    while lastB % 4 != 0:
        padb += 16
        lastB = len(groups[-1]) * TB + padb
    n_sub = n_sub_main + padb
    Ep = n_sub * P

    # per-subtile chunk binding + gather call table (same for all cores).
    # A spans get one call per chunk, so per-chunk padding is trailing and
    # the SWDGE generator can skip it (negative idxs); B spans get one call
    # per group, split into <= CALLSUB pieces (multiples of 4 subtiles).
    CALLSUB = min(32, max(TA, 2 * TB))
    CALLSUB = (CALLSUB // 4) * 4
    st_chunk = np.zeros(n_sub, dtype=np.int64)
    calls = []            # (start_subtile, n_subtiles, half)  half: 0=A 1=B
    s = 0
    span_start = {}
    for gi, g in enumerate(groups):
        for c in g:
            span_start[(c, 0)] = s
            st_chunk[s:s + TA] = c
            off = 0
            while off < TA:
                take = min(CALLSUB, TA - off)
                calls.append((s + off, take, 0))
                off += take
            s += TA
        b0 = s
        for c in g:
            span_start[(c, 1)] = s
            st_chunk[s:s + TB] = c
            s += TB
        if gi == len(groups) - 1 and padb:
            st_chunk[s:s + padb] = g[-1]
            s += padb
        blen = s - b0
        off = 0
        while off < blen:
            take = min(CALLSUB, blen - off)
            calls.append((b0 + off, take, 1))
            off += take
    assert s == n_sub
    max_call_sub = max(ns for _, ns, _ in calls)
    assert all(ns % 4 == 0 for _, ns, _ in calls)

    # last subtile index of each chunk (for psum close / stop flag)
    chunk_last = np.zeros(K, dtype=np.int64)
    chunk_first = np.zeros(K, dtype=np.int64)
    seen = set()
    for st in range(n_sub):
        c = st_chunk[st]
        if c not in seen:
            chunk_first[c] = st
            seen.add(c)
        chunk_last[c] = st

    # ---- per-core edge data -------------------------------------------------
    src_new = new_of_old[src]
    dst_new = new_of_old[dst]
    e_core = dst_new // SH
    e_chunk = (dst_new % SH) // P
    e_half = (src_new >= a_cap).astype(np.int64)

    idx_lin = np.full((n_cores, Ep), -1 if NEG_IDX else 0, dtype=np.int16)
    dstrel_lin = np.full((n_cores, Ep), -1.0, dtype=np.float32)
    osrc_lin = np.zeros((n_cores, Ep), dtype=np.int64)
    odst_lin = np.zeros((n_cores, Ep), dtype=np.int64)

    # bucket edges by (core, chunk, half)
    order = np.lexsort((e_half, e_chunk, e_core))
    so_src, so_dst = src_new[order], dst_new[order]
    so_core, so_chunk, so_half = e_core[order], e_chunk[order], e_half[order]
    so_osrc, so_odst = src[order], dst[order]
    # boundaries
    keys = so_core * (K * 2) + so_chunk * 2 + so_half
    bstart = np.searchsorted(keys, np.arange(n_cores * K * 2), side="left")
    bend = np.searchsorted(keys, np.arange(n_cores * K * 2), side="right")

    for core in range(n_cores):
        for c in range(K):
            for h in (0, 1):
                bi = core * (K * 2) + c * 2 + h
                e0, e1 = bstart[bi], bend[bi]
                cnt = e1 - e0
                cap = (TA if h == 0 else TB) * P
                assert cnt <= cap, (core, c, h, cnt, cap)
                p0 = span_start[(c, h)] * P
                sl = slice(p0, p0 + cnt)
                if h == 0:
                    idx_lin[core, sl] = so_src[e0:e1].astype(np.int16)
                else:
                    idx_lin[core, sl] = (so_src[e0:e1] - b_base).astype(np.int16)
                dstrel_lin[core, sl] = (so_dst[e0:e1] % P).astype(np.float32)
                osrc_lin[core, sl] = so_osrc[e0:e1]
                odst_lin[core, sl] = so_odst[e0:e1]

    # interior padding (pad rows followed by a real row within the same
    # call) must gather a safe row (0); only trailing pads stay -1 so the
    # SWDGE descriptor generator drops them.
    # Per-call gather count, uniform across cores (SPMD): the value-trim in
    # the SWDGE ucode and the ring-space reservation in the decode stage must
    # agree, and the decode uses num_idxs_reg.  cnt = max real rows over
    # cores (16-rounded); pads below cnt gather row 0, rows >= cnt are -1 and
    # are skipped by every core identically.
    call_cnt = []
    for (st0, nsx, half) in calls:
        r0, r1 = st0 * P, (st0 + nsx) * P
        cnt = 0
        for core in range(n_cores):
            real = np.nonzero(dstrel_lin[core, r0:r1] >= 0)[0]
            if len(real):
                cnt = max(cnt, int(real[-1]) + 1)
        cnt = min((cnt + 15) // 16 * 16, nsx * P)
        call_cnt.append(cnt)
        if NEG_IDX:
            for core in range(n_cores):
                seg = idx_lin[core, r0:r0 + cnt]
                seg[dstrel_lin[core, r0:r0 + cnt] < 0] = 0
            idx_lin[:, r0 + cnt:r1] = -1
        else:
            call_cnt[-1] = nsx * P
            for core in range(n_cores):
                seg = idx_lin[core, r0:r1]
                seg[dstrel_lin[core, r0:r1] < 0] = 0

    pl.n_atoms, pl.n_edges, pl.npad = n_atoms, n_edges, npad
    pl.n_cores, pl.K, pl.SH, pl.Ep, pl.n_sub = n_cores, K, SH, Ep, n_sub
    pl.TA, pl.TB, pl.padb = TA, TB, padb
    pl.a_cap, pl.b_base = a_cap, b_base
    pl.groups, pl.calls, pl.max_call_sub = groups, calls, max_call_sub
    pl.call_cnt = call_cnt
    pl.st_chunk, pl.chunk_first, pl.chunk_last = st_chunk, chunk_first, chunk_last
    pl.new_of_old, pl.old_of_new = new_of_old, old_of_new
    pl.idx_lin, pl.dstrel_lin = idx_lin, dstrel_lin
    pl.osrc_lin, pl.odst_lin = osrc_lin, odst_lin
    pl.n_per = int(n_per)
    pl.n_mol = n_atoms // pl.n_per
    return pl


def make_inputs(pl, r, xyz, a, embed, weights):
    """Build per-core in_maps.  weights: dict of raw weight arrays."""
    C, K, SH, Ep, n_sub = pl.n_cores, pl.K, pl.SH, pl.Ep, pl.n_sub
    NC = weights["fw1"].shape[0]
    NM = pl.n_mol
    F0 = Ep // P

    h0_all = embed[r[:, 0].astype(np.int64)].astype(np.float32)     # [n,NB]
    h0_new = np.zeros((pl.npad, NB), dtype=np.float32)
    real = pl.old_of_new < pl.n_atoms
    h0_new[real] = h0_all[pl.old_of_new[real]]

    mol_new = np.full(pl.npad, -1, dtype=np.int64)
    mol_new[real] = pl.old_of_new[real] // pl.n_per

    xyzf = xyz.astype(np.float32)

    fw1, fb1 = weights["fw1"], weights["fb1"]
    fw2, fb2 = weights["fw2"], weights["fb2"]
    afw, afb = weights["afw"], weights["afb"]
    ow1, ob1 = weights["ow1"], weights["ob1"]
    ow2, ob2 = weights["ow2"], weights["ob2"]
    aw1, ab1 = weights["aw1"], weights["ab1"]
    aw2, ab2 = weights["aw2"], weights["ab2"]
    assert np.all(afb == 0.0), "nonzero afb not supported by this kernel"

    # fold ssp's -log(2) into the following layer's bias
    fb2e = (fb2 - LN2 * fw2.sum(axis=1)).astype(np.float32)         # [NC,NB]
    ob2e = (ob2 - LN2 * ow2.sum(axis=1)).astype(np.float32)         # [NC,NB]
    ab2e = float(ab2[0] - LN2 * aw2.sum(axis=0)[0])

    offs = np.linspace(0.0, 5.0, NG).astype(np.float32)
    width = float(offs[1] - offs[0])
    coeff = -0.5 / (width * width)

    # fb2 replicated across partitions: [P, NC*4*NB]
    fb2rep = np.concatenate([np.tile(fb2e[i], (P, 4)) for i in range(NC)],
                            axis=1).astype(np.float32)

    shared = {
        "fw1b": np.ascontiguousarray(
            fw1.transpose(1, 0, 2).reshape(NG, NC * NB)).astype(ml_dtypes.bfloat16),
        "fw2b": np.ascontiguousarray(
            fw2.transpose(1, 0, 2).reshape(NB, NC * NB)).astype(ml_dtypes.bfloat16),
        "afwb": np.ascontiguousarray(
            afw.transpose(1, 0, 2).reshape(NB, NC * NB)).astype(ml_dtypes.bfloat16),
        "ow1w": np.ascontiguousarray(
            ow1.transpose(1, 0, 2).reshape(NB, NC * NB)).astype(np.float32),
        "ow2w": np.ascontiguousarray(
            ow2.transpose(1, 0, 2).reshape(NB, NC * NB)).astype(np.float32),
        "aw1w": aw1.astype(np.float32),                              # [NB,NH]
        "aw2w": aw2.astype(np.float32),                              # [NH,1]
        "fb1t": np.ascontiguousarray(fb1.T).astype(np.float32),      # [NB,NC]
        "ob1t": np.ascontiguousarray(ob1.T).astype(np.float32),
        "ob2et": np.ascontiguousarray(ob2e.T).astype(np.float32),
        "fb2rep": fb2rep,                                            # [P,NC*512]
        "ab1t": ab1.reshape(NH, 1).astype(np.float32),
        "ab2p": np.full((P, 1), ab2e, dtype=np.float32),
        "negmu": np.tile(-offs, 4).reshape(P, 1).astype(np.float32),
        "iota512": np.tile(np.arange(P, dtype=np.float32), (P, 4)),
    }

    in_maps = []
    for c in range(C):
        m = dict(shared)
        osrc = pl.osrc_lin[c]
        odst = pl.odst_lin[c]
        xs = xyzf[osrc]          # [Ep,3]
        xd = xyzf[odst]
        m["xsx"] = np.ascontiguousarray(xs[:, 0].reshape(P, F0))
        m["xsy"] = np.ascontiguousarray(xs[:, 1].reshape(P, F0))
        m["xsz"] = np.ascontiguousarray(xs[:, 2].reshape(P, F0))
        m["xdx"] = np.ascontiguousarray(xd[:, 0].reshape(P, F0))
        m["xdy"] = np.ascontiguousarray(xd[:, 1].reshape(P, F0))
        m["xdz"] = np.ascontiguousarray(xd[:, 2].reshape(P, F0))
        m["idx"] = np.ascontiguousarray(
            np.tile(pl.idx_lin[c].reshape(Ep // 16, 16).T, (8, 1)))
        m["dstrel"] = np.ascontiguousarray(
            pl.dstrel_lin[c].reshape(n_sub, P).T)
        m["h0t"] = np.ascontiguousarray(
            h0_new[c * SH:(c + 1) * SH].T)                          # [NB,SH]
        msk = np.zeros((K, P, NM), dtype=np.float32)
        mols = mol_new[c * SH:(c + 1) * SH].reshape(K, P)
        for mm in range(NM):
            msk[:, :, mm] = (mols == mm)
        m["mask"] = msk
        in_maps.append(m)
    return in_maps, coeff


# ----------------------------------------------------------------------------
# Device program
# ----------------------------------------------------------------------------

def _ap(tile_ap, extra_off, pattern):
    """Raw access-pattern surgery on a (pool-tile or dram) AP."""
    return bass.AP(tile_ap.tensor, tile_ap.offset + extra_off, pattern)


def _patch_act_tables():
    """Pin each activation function to exactly one ACT table so bacc never
    thrashes table loads: Softplus/Copy/Identity -> softplus_and_others
    (Softplus is missing from act_info's listing but present in the HW
    table), Exp -> exp_and_others, Sqrt -> sqrt_and_others."""
    if getattr(bacc, "_act_tables_patched", False):
        return
    orig = bacc.get_activation_tables

    if USE_SOFTPLUS:
        def patched(arch):
            t = dict(orig(arch))
            shared = {AF.Identity, AF.Copy, AF.Square}
            for name in list(t):
                s = set(t[name])
                if name == "softplus_and_others":
                    s |= {AF.Softplus}
                else:
                    s -= shared | {AF.Softplus}
                if name != "exp_and_others":
                    s -= {AF.Exp}
                if name != "sqrt_and_others":
                    s -= {AF.Sqrt}
                t[name] = s
            return t
    else:
        def patched(arch):
            t = dict(orig(arch))
            shared = {AF.Exp, AF.Ln, AF.Identity, AF.Copy, AF.Square}
            for name in list(t):
                if name != "natural_log_exp_and_others":
                    t[name] = t[name] - shared
            return t

    bacc.get_activation_tables = patched
    bacc._act_tables_patched = True


def build_program(pl, NC, NM, coeff):
    _patch_act_tables()
    C, K, SH, Ep, n_sub = pl.n_cores, pl.K, pl.SH, pl.Ep, pl.n_sub
    F0 = Ep // P
    Q = Ep // 4                      # edges per gaussian partition-group
    NW = 4                           # phase-0 g-build col iterations
    while Q % NW != 0 or (Q // NW) > 1024:
        NW *= 2
    Wg = Q // NW
    CS = pl.max_call_sub

    nc = bacc.Bacc("TRN2", target_bir_lowering=False, debug=False,
                   enable_asserts=False, num_devices=C, num_swdge_queues=4,
                   dynamic_dma_scratch_size=int(os.environ.get("DMA_SCRATCH", "16384")))

    def din(name, shape, dt=F32):
        return nc.dram_tensor(name, shape, dt, kind="ExternalInput").ap()

    xsx, xsy, xsz = din("xsx", [P, F0]), din("xsy", [P, F0]), din("xsz", [P, F0])
    xdx, xdy, xdz = din("xdx", [P, F0]), din("xdy", [P, F0]), din("xdz", [P, F0])
    idx_d = din("idx", [P, Ep // 16], I16)
    dstrel_d = din("dstrel", [P, n_sub])
    h0t_d = din("h0t", [NB, SH])
    mask_d = din("mask", [K, P, NM])
    fw1b_d = din("fw1b", [NG, NC * NB], BF16)
    fw2b_d = din("fw2b", [NB, NC * NB], BF16)
    afwb_d = din("afwb", [NB, NC * NB], BF16)
    ow1w_d = din("ow1w", [NB, NC * NB])
    ow2w_d = din("ow2w", [NB, NC * NB])
    aw1w_d = din("aw1w", [NB, NH])
    aw2w_d = din("aw2w", [NH, 1])
    fb1t_d = din("fb1t", [NB, NC])
    ob1t_d = din("ob1t", [NB, NC])
    ob2et_d = din("ob2et", [NB, NC])
    fb2rep_d = din("fb2rep", [P, NC * 4 * NB])
    ab1t_d = din("ab1t", [NH, 1])
    ab2p_d = din("ab2p", [P, 1])
    negmu_d = din("negmu", [P, 1])
    iota512_d = din("iota512", [P, 4 * P])

    ypart = nc.dram_tensor("ypart", [1, NM], F32, kind="ExternalOutput").ap()

    with tile.TileContext(nc) as tc:
        with ExitStack() as ctx:
            dram = ctx.enter_context(tc.tile_pool(name="dram", bufs=1, space="DRAM"))
            res = ctx.enter_context(tc.tile_pool(name="res", bufs=1))
            sb = ctx.enter_context(tc.tile_pool(name="sb", bufs=3))
            gpool = ctx.enter_context(tc.tile_pool(name="gpool", bufs=5))
            p0 = ctx.enter_context(tc.tile_pool(name="p0", bufs=2))
            spool = ctx.enter_context(tc.tile_pool(name="spool", bufs=3))
            pps = ctx.enter_context(tc.tile_pool(name="pps", bufs=2, space="PSUM"))
            ppagg = ctx.enter_context(tc.tile_pool(name="ppagg", bufs=2, space="PSUM"))
            ppu = ctx.enter_context(tc.tile_pool(name="ppu", bufs=2, space="PSUM"))

            # ---- DRAM scratch ----
            d_dram = dram.tile([P, F0], F32)
            g_dram = dram.tile([4, NG, Q], BF16)
            ag_space = "Shared" if SHARED_AG else "Local"
            hf_my = [dram.tile([SH, NB], BF16, name=f"hf_my{i}")
                     for i in range(NC)]
            hf_tab = [dram.tile([pl.npad, NB], BF16, addr_space=ag_space,
                                name=f"hf_tab{i}")
                      for i in range(NC)]

            # ---- resident SBUF ----
            h_my = res.tile([NB, SH], F32)
            agg_sb = res.tile([NB, SH], F32)
            idx_sb = res.tile([P, Ep // 16], I16)
            dstrel_sb = res.tile([P, n_sub], F32)
            iota_sb = res.tile([P, 4 * P], F32)
            mask_sb = res.tile([P, K * NM], F32)
            fw1b_sb = res.tile([NG, NC * NB], BF16)
            fw2b_sb = res.tile([NB, NC * NB], BF16)
            afwb_sb = res.tile([NB, NC * NB], BF16)
            ow1_sb = res.tile([NB, NC * NB], F32)
            ow2_sb = res.tile([NB, NC * NB], F32)
            aw1_sb = res.tile([NB, NH], F32)
            aw2_sb = res.tile([NH, 1], F32)
            fb1_sb = res.tile([NB, NC], F32)
            ob1_sb = res.tile([NB, NC], F32)
            ob2e_sb = res.tile([NB, NC], F32)
            fb2rep_sb = res.tile([P, NC * 4 * NB], F32)
            ab1_sb = res.tile([NH, 1], F32)
            ab2p_sb = res.tile([P, 1], F32)
            negmu_sb = res.tile([P, 1], F32)
            epsb_sb = res.tile([P, 1], F32)
            e_acc = res.tile([1, NM], F32)
            nc.vector.memset(epsb_sb[:], EPS)
            nc.vector.memset(e_acc[:], 0.0)

            nc.sync.dma_start(h_my[:], h0t_d[:])
            nc.sync.dma_start(idx_sb[:], idx_d[:])
            nc.sync.dma_start(dstrel_sb[:], dstrel_d[:])
            nc.sync.dma_start(iota_sb[:], iota512_d[:])
            # mask [K,P,NM] -> [P, K*NM]
            nc.sync.dma_start(
                _ap(mask_sb[:], 0, [[K * NM, P], [NM, K], [1, NM]]),
                _ap(mask_d, 0, [[NM, P], [P * NM, K], [1, NM]]))
            for t_sb, t_d in [(fw1b_sb, fw1b_d), (fw2b_sb, fw2b_d),
                              (afwb_sb, afwb_d), (ow1_sb, ow1w_d),
                              (ow2_sb, ow2w_d), (aw1_sb, aw1w_d),
                              (aw2_sb, aw2w_d), (fb1_sb, fb1t_d),
                              (ob1_sb, ob1t_d), (ob2e_sb, ob2et_d),
                              (fb2rep_sb, fb2rep_d), (ab1_sb, ab1t_d),
                              (ab2p_sb, ab2p_d), (negmu_sb, negmu_d)]:
                nc.sync.dma_start(t_sb[:], t_d[:])

            # zero-init gather buffers once so skipped (trailing-pad) rows
            # always hold finite values
            for z in range(5):
                gz = gpool.tile([P, CS * NB], BF16, tag="gbuf", name=f"gz{z}")
                nc.vector.memset(gz[:], 0.0)
            prep_bufs = []
            for z in range(PREP_N):
                gp = gpool.tile([P, CS * NB], BF16, tag=f"gp{z}", bufs=1,
                                name=f"gp{z}")
                nc.vector.memset(gp[:], 0.0)
                prep_bufs.append(gp)

            def emit_hf_chunk(i, c):
                """hf rows for chunk c of conv i from current h_my."""
                hb = sb.tile([NB, P], BF16, tag="hb")
                if int(os.environ.get("HB_SCALAR", "1")):
                    nc.scalar.copy(hb[:], h_my[:, P * c:P * (c + 1)])
                else:
                    nc.vector.tensor_copy(hb[:], h_my[:, P * c:P * (c + 1)])
                hfps = ppu.tile([P, P], F32, tag="upd", name=f"hfps_{i}_{c}")
                nc.tensor.matmul(hfps[:], hb[:],
                                 afwb_sb[:, NB * i:NB * (i + 1)],
                                 start=True, stop=True)
                hfsb = sb.tile([P, P], BF16, tag="hfsb")
                nc.scalar.copy(hfsb[:], hfps[:])
                nc.sync.dma_start(hf_my[i][P * c:P * (c + 1), :], hfsb[:])

            def emit_update_chunk(i, c):
                """h += dense(ssp(dense(agg)))  for chunk c, conv i."""
                ups = ppu.tile([P, P], F32, tag="upd", name=f"ups_{i}_{c}")
                nc.tensor.matmul(ups[:], ow1_sb[:, NB * i:NB * (i + 1)],
                                 agg_sb[:, P * c:P * (c + 1)],
                                 start=True, stop=True)
                usb = sb.tile([P, P], F32, tag="usb")
                if USE_SOFTPLUS:
                    nc.scalar.activation(usb[:], ups[:], AF.Softplus,
                                         bias=ob1_sb[:, i:i + 1], scale=1.0)
                else:
                    ue = sb.tile([P, P], F32, tag="ue")
                    nc.scalar.activation(ue[:], ups[:], AF.Exp,
                                         bias=ob1_sb[:, i:i + 1], scale=1.0)
                    nc.scalar.activation(usb[:], ue[:], AF.Ln,
                                         bias=1.0, scale=1.0)
                drps = ppu.tile([P, P], F32, tag="upd", name=f"drps_{i}_{c}")
                nc.tensor.matmul(drps[:], ow2_sb[:, NB * i:NB * (i + 1)],
                                 usb[:], start=True, stop=True)
                drt = sb.tile([P, P], F32, tag="drt")
                nc.vector.tensor_scalar(drt[:], drps[:],
                                        ob2e_sb[:, i:i + 1], None,
                                        op0=OP.add)
                nc.vector.tensor_add(h_my[:, P * c:P * (c + 1)],
                                     h_my[:, P * c:P * (c + 1)], drt[:])

            def emit_readout_chunk(c):
                r1ps = ppu.tile([NH, P], F32, tag="upd", name=f"r1ps{c}")
                nc.tensor.matmul(r1ps[:], aw1_sb[:],
                                 h_my[:, P * c:P * (c + 1)],
                                 start=True, stop=True)
                r1sb = sb.tile([NH, P], F32, tag="r1sb")
                if USE_SOFTPLUS:
                    nc.scalar.activation(r1sb[:], r1ps[:], AF.Softplus,
                                         bias=ab1_sb[:, 0:1], scale=1.0)
                else:
                    r1e = sb.tile([NH, P], F32, tag="r1e")
                    nc.scalar.activation(r1e[:], r1ps[:], AF.Exp,
                                         bias=ab1_sb[:, 0:1], scale=1.0)
                    nc.scalar.activation(r1sb[:], r1e[:], AF.Ln,
                                         bias=1.0, scale=1.0)
                yps = ppu.tile([P, 1], F32, tag="upd", name=f"yps{c}")
                nc.tensor.matmul(yps[:], r1sb[:], aw2_sb[:],
                                 start=True, stop=True)
                ysb = sb.tile([P, 1], F32, tag="ysb")
                nc.scalar.activation(ysb[:], yps[:], AF.Identity,
                                     bias=ab2p_sb[:, 0:1], scale=1.0)
                em_ps = ppu.tile([1, NM], F32, tag="upd", name=f"emps{c}")
                nc.tensor.matmul(em_ps[:], ysb[:],
                                 mask_sb[:, NM * c:NM * (c + 1)],
                                 start=True, stop=True)
                nc.vector.tensor_add(e_acc[:], e_acc[:], em_ps[:])

            # startup: hf table for conv 0 from h0
            for c in range(K):
                emit_hf_chunk(0, c)

            # ================= phase 0: distances and gaussians ============
            cx = p0.tile([P, F0], F32, tag="ph0", bufs=1)
            cy = p0.tile([P, F0], F32, tag="ph0b", bufs=1)
            cz = p0.tile([P, F0], F32, tag="ph0c", bufs=1)
            tx = p0.tile([P, F0], F32, tag="ph0d", bufs=1)
            nc.sync.dma_start(cx[:], xsx[:])
            nc.sync.dma_start(tx[:], xdx[:])
            nc.vector.tensor_sub(cx[:], cx[:], tx[:])
            nc.vector.tensor_mul(cx[:], cx[:], cx[:])
            nc.sync.dma_start(cy[:], xsy[:])
            nc.sync.dma_start(tx[:], xdy[:])
            nc.vector.tensor_sub(cy[:], cy[:], tx[:])
            nc.vector.tensor_mul(cy[:], cy[:], cy[:])
            nc.sync.dma_start(cz[:], xsz[:])
            nc.sync.dma_start(tx[:], xdz[:])
            nc.vector.tensor_sub(cz[:], cz[:], tx[:])
            nc.vector.tensor_mul(cz[:], cz[:], cz[:])
            nc.vector.tensor_add(cx[:], cx[:], cy[:])
            nc.vector.tensor_add(cx[:], cx[:], cz[:])
            nc.scalar.activation(cy[:], cx[:], AF.Sqrt,
                                 bias=epsb_sb[:, 0:1], scale=1.0)
            nc.sync.dma_start(d_dram[:], cy[:])

            for w in range(NW):
                dbc = p0.tile([P, Wg], F32, tag="dbc")
                nc.sync.dma_start(
                    dbc[:], _ap(d_dram[:], w * Wg, [[Q, 4], [0, NG], [1, Wg]]))
                t1 = p0.tile([P, Wg], F32, tag="t1")
                nc.vector.tensor_scalar(t1[:], dbc[:], negmu_sb[:, 0:1], None,
                                        op0=OP.add)
                nc.vector.tensor_mul(t1[:], t1[:], t1[:])
                gt = p0.tile([P, Wg], BF16, tag="gt0")
                nc.scalar.activation(gt[:], t1[:], AF.Exp, bias=0.0, scale=coeff)
                nc.sync.dma_start(
                    _ap(g_dram[:], w * Wg, [[NG * Q, 4], [Q, NG], [1, Wg]]),
                    gt[:])

            # ================= conv layers =================================
            for i in range(NC):
                # --- allgather hf shards into the full table ------------
                nc.gpsimd.collective_compute(
                    "AllGather", OP.bypass,
                    replica_groups=[list(range(C))],
                    ins=[hf_my[i].opt()], outs=[hf_tab[i].opt()])

                # --- edge phase ---------------------------------------
                if i > 0 and PREP_N:
                    # fire the descriptors pre-generated during conv i-1
                    for qn in range(min(PREP_N, 4)):
                        nc.gpsimd.trigger_dma(count=None, queue_num=qn)
                agg_open = {}
                for ci, (st0, nsx, half) in enumerate(pl.calls):
                    if half == 0:
                        tbl_ap = _ap(hf_tab[i][:], 0,
                                     [[NB, pl.a_cap], [1, NB]])
                    else:
                        tbl_ap = _ap(hf_tab[i][:], pl.b_base * NB,
                                     [[NB, pl.npad - pl.b_base], [1, NB]])
                    if i > 0 and ci < PREP_N:
                        gbuf = prep_bufs[ci]    # gathered via prep+trigger
                    else:
                        gbuf = gpool.tile([P, CS * NB], BF16, tag="gbuf")
                        nc.gpsimd.dma_gather(
                            _ap(gbuf[:], 0,
                                [[CS * NB, P], [NB, nsx], [1, NB]]),
                            tbl_ap,
                            idx_sb[:, 8 * st0:8 * (st0 + nsx)],
                            P * nsx, pl.call_cnt[ci], NB, single_packet=False)

                    for b in range(nsx // 4):
                        stb = st0 + 4 * b
                        e0 = stb * P
                        q, col = e0 // Q, e0 % Q
                        gt2 = sb.tile([NG, 512], BF16, tag="gt2")
                        nc.sync.dma_start(gt2[:], g_dram[q, :, col:col + 512])
                        ps1 = pps.tile([P, 512], F32, tag="ps1")
                        nc.tensor.matmul(ps1[:],
                                         fw1b_sb[:, NB * i:NB * (i + 1)],
                                         gt2[:], start=True, stop=True)
                        x1 = sb.tile([P, 512], BF16, tag="x1")
                        if USE_SOFTPLUS:
                            nc.scalar.activation(x1[:], ps1[:], AF.Softplus,
                                                 bias=fb1_sb[:, i:i + 1],
                                                 scale=1.0)
                        else:
                            nc.scalar.activation(ps1[:], ps1[:], AF.Exp,
                                                 bias=fb1_sb[:, i:i + 1],
                                                 scale=1.0)
                            nc.scalar.activation(x1[:], ps1[:], AF.Ln,
                                                 bias=1.0, scale=1.0)
                        ps2 = pps.tile([P, 512], F32, tag="ps2")
                        if PREFILL:
                            nc.scalar.copy(
                                ps2[:], fb2rep_sb[:, 512 * i:512 * (i + 1)])
                        for s4 in range(4):
                            nc.tensor.matmul(
                                ps2[:, P * s4:P * (s4 + 1)],
                                x1[:, P * s4:P * (s4 + 1)],
                                fw2b_sb[:, NB * i:NB * (i + 1)],
                                start=not PREFILL, stop=True,
                                skip_group_check=True)
                        Sm = spool.tile([P, 512], BF16, tag="Sm")
                        dr_ap = _ap(dstrel_sb[:], stb,
                                    [[n_sub, P], [1, 4], [0, P]])
                        nc.vector.tensor_tensor(Sm[:], iota_sb[:], dr_ap,
                                                op=OP.is_equal)
                        msg = sb.tile([P, 512], BF16, tag="msg")
                        if PREFILL:
                            hfg = _ap(gbuf[:], 4 * b * NB,
                                      [[CS * NB, P], [1, 512]])
                            nc.vector.tensor_tensor(msg[:], ps2[:], hfg,
                                                    op=OP.mult)
                        else:
                            tmp = sb.tile([P, 512], F32, tag="tmpb")
                            nc.vector.tensor_tensor(
                                tmp[:], ps2[:],
                                fb2rep_sb[:, 512 * i:512 * (i + 1)],
                                op=OP.add)
                            hfg = _ap(gbuf[:], 4 * b * NB,
                                      [[CS * NB, P], [1, 512]])
                            nc.vector.tensor_tensor(msg[:], tmp[:], hfg,
                                                    op=OP.mult)
                        for s4 in range(4):
                            st = stb + s4
                            cki = int(pl.st_chunk[st])
                            if cki not in agg_open:
                                agg_open[cki] = ppagg.tile(
                                    [P, P], F32, tag="agg",
                                    name=f"aggps_{i}_{cki}")
                            first = (st == pl.chunk_first[cki])
                            last = (st == pl.chunk_last[cki])
                            nc.tensor.matmul(
                                agg_open[cki][:],
                                msg[:, P * s4:P * (s4 + 1)],
                                Sm[:, P * s4:P * (s4 + 1)],
                                start=first, stop=last,
                                skip_group_check=True)
                            if last:
                                nc.vector.tensor_copy(
                                    agg_sb[:, P * cki:P * (cki + 1)],
                                    agg_open[cki][:])
                                del agg_open[cki]
                                emit_update_chunk(i, cki)
                                if i + 1 < NC:
                                    emit_hf_chunk(i + 1, cki)
                                else:
                                    emit_readout_chunk(cki)

                # pre-generate descriptors for conv i+1's first calls while
                # the chunk tail + AllGather drain (DMA fires at trigger)
                if i + 1 < NC and PREP_N:
                    for ci in range(PREP_N):
                        st0, nsx, half = pl.calls[ci]
                        if half == 0:
                            ntbl = _ap(hf_tab[i + 1][:], 0,
                                       [[NB, pl.a_cap], [1, NB]])
                        else:
                            ntbl = _ap(hf_tab[i + 1][:], pl.b_base * NB,
                                       [[NB, pl.npad - pl.b_base], [1, NB]])
                        psem = nc.alloc_semaphore(f"prep_{i + 1}_{ci}")
                        nc.gpsimd.dma_gather(
                            _ap(prep_bufs[ci][:], 0,
                                [[CS * NB, P], [NB, nsx], [1, NB]]),
                            ntbl,
                            idx_sb[:, 8 * st0:8 * (st0 + nsx)],
                            P * nsx, pl.call_cnt[ci], NB,
                            single_packet=False, prepare_only=True,
                            sem=psem, queue_num=ci % 4)

            nc.sync.dma_start(ypart[:], e_acc[:])

    # Spread gather descriptor-generation across the 4 SWDGE queues (Q7
    # core pairs), consistent with the DMASW semaphore lane Tile assigned
    # (the runtime locks each DMA semaphore to one SWDGE queue).
    import concourse.tile_sem_assignment as tsa
    sw_procs = {tsa.PROC_NAME_TO_IDX[f"DMASW{k}"]: k for k in range(8)}
    locked0 = set()
    gathers = []
    for b in nc.main_func.blocks:
        for inst in b.instructions:
            proc = getattr(inst, "bass_scheduled_proc", None)
            if proc in sw_procs:
                if isinstance(inst, mybir.InstDMAGatherAnt):
                    if getattr(inst, "gen_mode", 0) != 1:
                        gathers.append((inst, sw_procs[proc]))
                else:
                    locked0.add(sw_procs[proc])
    for inst, lane in gathers:
        inst.queue_num = 0 if lane in locked0 else lane % 4

    nc.compile()
    return nc


# ----------------------------------------------------------------------------
# Entry point
# ----------------------------------------------------------------------------

_CACHE = {}


def _get_program(pl, NC, NM, coeff):
    key = (pl.n_atoms, pl.n_edges, pl.Ep, pl.K, NC, NM, round(coeff, 9))
    if key not in _CACHE:
        _CACHE[key] = build_program(pl, NC, NM, coeff)
    return _CACHE[key]


def kernel(r, xyz, a, n_per, embed, fw1, fb1, fw2, fb2, afw, afb,
           ow1, ob1, ow2, ob2, aw1, ab1, aw2, ab2, trace=False):
    r = np.asarray(r)
    xyz = np.asarray(xyz, dtype=np.float32)
    a = np.asarray(a)
    weights = dict(fw1=np.asarray(fw1), fb1=np.asarray(fb1),
                   fw2=np.asarray(fw2), fb2=np.asarray(fb2),
                   afw=np.asarray(afw), afb=np.asarray(afb),
                   ow1=np.asarray(ow1), ob1=np.asarray(ob1),
                   ow2=np.asarray(ow2), ob2=np.asarray(ob2),
                   aw1=np.asarray(aw1), ab1=np.asarray(ab1),
                   aw2=np.asarray(aw2), ab2=np.asarray(ab2))
    pl = make_plan(r, xyz, a, int(n_per), n_cores=8)
    in_maps, coeff = make_inputs(pl, r, xyz, a, np.asarray(embed), weights)
    NC = weights["fw1"].shape[0]
    nc = _get_program(pl, NC, pl.n_mol, coeff)
    res = bass_utils.run_bass_kernel_spmd(
        nc, in_maps, core_ids=list(range(pl.n_cores)), trace=trace)
    out = np.zeros(pl.n_mol, dtype=np.float64)
    for k in range(pl.n_cores):
        out += res.results[k]["ypart"][0].astype(np.float64)
    kernel._last_results = res
    return out.astype(np.float32)
